# revision 1
# baseline (speedup 1.0000x reference)
"""Distributed Trainium2 kernel for nn_Attention_61332132987140.

Gated multi-head attention block: RMSNorm -> QKV proj -> RoPE -> softmax
attention -> sigmoid head gating -> output projection.

Sharding: 8 cores = 2 batch groups x 4-head groups (tensor parallel on
heads within a batch). Each core computes attention for its batch's full
sequence over its 4 heads, the partial output projection over its 256
columns of w_out, then a ReduceScatter over its 4-core batch group sums
the partials and leaves each core with a disjoint 128-token slice per
512-token quarter. The host reassembles the full (2, 2048, 1024) output.

Device compute dtype: bf16 operands into the PE array with fp32 PSUM
accumulation; softmax/normalization math in fp32 on ACT/DVE.
"""
import os
import sys

sys.path.insert(0, "/opt/trn_rl_repo")

import numpy as np
import ml_dtypes

import concourse.bass as bass
import concourse.mybir as mybir
import concourse.tile as tile
from concourse import bacc
from concourse.bass_utils import run_bass_kernel_spmd

F32 = mybir.dt.float32
BF16 = mybir.dt.bfloat16
AF = mybir.ActivationFunctionType
ALU = mybir.AluOpType

B, N, DIM = 2, 2048, 1024
HEADS, DH = 16, 64
HL = 4  # local heads per core
P = 128
TT = N // P  # 16 token tiles
KD = DIM // P  # 8 contraction tiles
NQ = 4  # quarters (512-token i-chunks)
QT = N // NQ
CORES = 8
REPLICA_GROUPS = [[0, 1, 2, 3], [4, 5, 6, 7]]

_nc_cache = None
_last_result = None


def _build():
    nc = bacc.Bacc("TRN2", target_bir_lowering=False, debug=False, num_devices=CORES)

    x_ext = nc.declare_dram_parameter("x", [N, DIM], F32, isOutput=False)
    wqkv_ext = nc.declare_dram_parameter("wqkv", [DIM, 3 * HL * DH], BF16, isOutput=False)
    wg_ext = nc.declare_dram_parameter("wg", [DIM, HL], BF16, isOutput=False)
    bgn_ext = nc.declare_dram_parameter("bgn", [HL, 1], F32, isOutput=False)
    wout_ext = nc.declare_dram_parameter("wout", [HL * DH, DIM], BF16, isOutput=False)
    cosr_ext = nc.declare_dram_parameter("cosr", [N, 512], BF16, isOutput=False)
    sinr_ext = nc.declare_dram_parameter("sinr", [N, 512], BF16, isOutput=False)
    out_ext = nc.declare_dram_parameter("out", [NQ, P, DIM], F32, isOutput=True)
    dbg = {}
    if os.environ.get("KDEBUG"):
        dbg["xnT"] = nc.declare_dram_parameter("dbg_xnT", [P, 4, KD, P], BF16,
                                               isOutput=True)
        dbg["qkt"] = nc.declare_dram_parameter("dbg_qkt", [P, TT, 4, P], BF16,
                                               isOutput=True)
        dbg["v"] = nc.declare_dram_parameter("dbg_v", [P, TT, HL * DH], BF16,
                                             isOutput=True)
        dbg["gates"] = nc.declare_dram_parameter("dbg_gates", [P, 512], F32,
                                                 isOutput=True)
        dbg["pt"] = nc.declare_dram_parameter("dbg_pt", [P, 2, 512], BF16,
                                              isOutput=True)
        dbg["sums"] = nc.declare_dram_parameter("dbg_sums", [P, 512], F32,
                                                isOutput=True)
        dbg["av"] = nc.declare_dram_parameter("dbg_av", [P, 512], F32,
                                              isOutput=True)
        dbg["oT"] = nc.declare_dram_parameter("dbg_oT", [P, 2, 512], BF16,
                                              isOutput=True)
        dbg["y"] = nc.declare_dram_parameter("dbg_y", [QT, DIM], F32,
                                             isOutput=True)

    with tile.TileContext(nc) as tc:
        with (
            tc.tile_pool(name="wpool", bufs=1) as wpool,
            tc.tile_pool(name="persist", bufs=1) as persist,
            tc.tile_pool(name="xstream", bufs=4) as xstream,
            tc.tile_pool(name="stream", bufs=3) as stream,
            tc.tile_pool(name="xntp", bufs=2) as xntp,
            tc.tile_pool(name="ptp", bufs=28) as ptp,
            tc.tile_pool(name="tail", bufs=2) as tailp,
            tc.tile_pool(name="ps_qa1", bufs=1, space="PSUM") as ps_qa1,
            tc.tile_pool(name="ps_qa2", bufs=1, space="PSUM") as ps_qa2,
            tc.tile_pool(name="ps_s", bufs=2, space="PSUM") as ps_s,
                        tc.tile_pool(name="ps_attn", bufs=1, space="PSUM") as ps_attn,
            tc.tile_pool(name="ps_ygs", bufs=2, space="PSUM") as ps_ygs,
            tc.tile_pool(name="dram", bufs=1, space="DRAM") as dramp,
        ):
            # ---- constants / weights ----
            wqkv_sb = wpool.tile([P, KD, 768], BF16)
            nc.scalar.dma_start(
                wqkv_sb[:], wqkv_ext.rearrange("(k p) f -> p k f", p=P)
            )
            wg_sb = wpool.tile([P, KD, HL], BF16)
            nc.scalar.dma_start(wg_sb[:], wg_ext.rearrange("(k p) f -> p k f", p=P))
            wout_sb = wpool.tile([P, 2, DIM], BF16)
            nc.scalar.dma_start(
                wout_sb[:], wout_ext.rearrange("(k p) f -> p k f", p=P)
            )
            bgn_sb = wpool.tile([HL, 1], F32)
            nc.scalar.dma_start(bgn_sb[:], bgn_ext[:])
            zb = wpool.tile([P, 1], F32)
            nc.gpsimd.memset(zb[:], 0.0)
            lb32 = wpool.tile([P, 1], F32)
            nc.gpsimd.memset(lb32[:], float(np.log(32.0)))
            ones_sb = wpool.tile([P, 1], BF16)
            nc.gpsimd.memset(ones_sb[:], 1.0)
            ones_row = wpool.tile([1, 64], BF16)
            nc.gpsimd.memset(ones_row[:], 1.0)

            # ---- persistent activations ----
            # QKT_sb[p, tok_tile, blk, t]: blk 0/1 = q head-pairs; 2/3 = k.
            # blk-last layout keeps each xbar-transpose destination dense
            QKT_sb = persist.tile([P, TT, 4, P], BF16)
            # v_sb[j_in_tile, jt, h*64+d]
            v_sb = persist.tile([P, TT, HL * DH], BF16)
            # gates for head h live at partition 32*h; other rows are garbage
            gates_sb = persist.tile([P, N], F32)

            def emit_se(ci, jt, dbg_tap=False):
                """scores + exp for one (quarter, j-tile); returns pt tiles"""
                pts = []
                for hp in range(2):
                    s_ps = ps_s.tile([P, 2, 512], F32, name="s_ps", tag="s")
                    for e in range(2):
                        nc.tensor.matmul(
                            s_ps[:, e, :],
                            QKT_sb[e * 64:(e + 1) * 64, jt, 2 + hp, :],
                            QKT_sb[e * 64:(e + 1) * 64, 4 * ci:4 * ci + 4,
                                   hp, :],
                            start=True, stop=True,
                        )
                    pt = ptp.tile([P, 2, 512], BF16, name="pt")
                    nc.scalar.activation(pt[:], s_ps[:], AF.Exp, scale=0.125,
                                         bias=zb[:])
                    pts.append(pt)
                    if dbg_tap and hp == 0:
                        nc.gpsimd.dma_start(dbg["pt"][:], pt[:])
                return pts

            def emit_avs(jt, pts, av01, av23, sums):
                """AV accumulation + softmax-sum matmuls for one j-tile"""
                for hp in range(2):
                    avt = av01 if hp == 0 else av23
                    # adjacent issue of col-disjoint AV matmuls -> concurrent.
                    # Concurrent accumulation groups in one bank are fine:
                    # partition ranges disjoint; has_written is per row
                    for e in range(2):
                        h = 2 * hp + e
                        nc.tensor.matmul(
                            avt[e * 64:(e + 1) * 64, :],
                            v_sb[:, jt, h * DH:(h + 1) * DH],
                            pts[hp][:, e, :],
                            start=(jt == 0), stop=(jt == TT - 1),
                            skip_group_check=True,
                        )
                # four sum matmuls back-to-back: distinct 32-col groups ->
                # one concurrent 512-cycle slot
                for hp in range(2):
                    for e in range(2):
                        h = 2 * hp + e
                        nc.tensor.matmul(
                            sums[h * 32:h * 32 + 1, :],
                            ones_sb[:, 0:1],
                            pts[hp][:, e, :],
                            start=(jt == 0), stop=(jt == TT - 1),
                            tile_position=(0, h * 32),
                            skip_group_check=True,
                        )

            # =========== Phase A: norm, QKV, RoPE, transposes ===========
            pre_pts = {}
            for ci in range(NQ):
                # xnT[p, tt, kd, t]: kd-last so each transpose dest is dense
                xnT = xntp.tile([P, 4, KD, P], BF16, name="xnT")
                # batch the 4 tiles' Ln/Exp into single ops: the ACT table
                # pass places a LoadActFuncSet at every Ln<->Exp alternation,
                # so grouping keeps it to 2 loads per chunk
                xts = []
                ss4 = stream.tile([P, 4], F32, name="ss4")
                for tt in range(4):
                    tok = ci * 4 + tt
                    x_t = xstream.tile([P, DIM], F32, name="x_t")
                    nc.gpsimd.dma_start(x_t[:], x_ext[tok * P:(tok + 1) * P, :])
                    xts.append(x_t)
                    scr = stream.tile([P, DIM], BF16, name="scr")
                    nc.vector.tensor_tensor(out=scr[:], in0=x_t[:], in1=x_t[:],
                                            op=ALU.mult)
                    nc.vector.reduce_sum(ss4[:, tt:tt + 1], scr[:],
                                         axis=mybir.AxisListType.X)
                ln4 = stream.tile([P, 4], F32, name="ln4")
                nc.scalar.activation(ln4[:], ss4[:], AF.Ln, bias=zb[:])
                sc4 = stream.tile([P, 4], F32, name="sc4")
                nc.scalar.activation(sc4[:], ln4[:], AF.Exp, scale=-0.5,
                                     bias=lb32[:])
                for tt in range(4):
                    xn_t = stream.tile([P, DIM], BF16, name="xn_t")
                    nc.scalar.activation(xn_t[:], xts[tt][:], AF.Copy,
                                         scale=sc4[:, tt:tt + 1])
                    nc.sync.dma_start_transpose(xnT[:, tt, :, :], xn_t[:])

                for tt in range(4):
                    tok = ci * 4 + tt
                    qk_ps = ps_qa1.tile([P, 512], F32, name="qk_ps", tag="qa1")
                    v_ps = ps_qa2.tile([P, 256], F32, name="v_ps", tag="qa2")
                    for kd in range(KD):
                        lhsT = xnT[:, tt, kd, :]
                        nc.tensor.matmul(qk_ps[:], lhsT,
                                         wqkv_sb[:, kd, 0:512],
                                         start=(kd == 0), stop=(kd == KD - 1))
                        nc.tensor.matmul(v_ps[:], lhsT,
                                         wqkv_sb[:, kd, 512:768],
                                         start=(kd == 0), stop=(kd == KD - 1))
                    # RoPE on q|k (psum cols 0:512), even/odd feature halves
                    cos_t = stream.tile([P, 512], BF16, name="cos_t")
                    nc.gpsimd.dma_start(cos_t[:], cosr_ext[tok * P:(tok + 1) * P, :])
                    sin_t = stream.tile([P, 512], BF16, name="sin_t")
                    nc.gpsimd.dma_start(sin_t[:], sinr_ext[tok * P:(tok + 1) * P, :])
                    qkv = qk_ps[:].rearrange("p (b c) -> p b c", b=8)
                    qE, qO = qkv[:, :, 0:32], qkv[:, :, 32:64]
                    cE = cos_t[:, 0:256].rearrange("p (b c) -> p b c", b=8)
                    cO = cos_t[:, 256:512].rearrange("p (b c) -> p b c", b=8)
                    sE = sin_t[:, 0:256].rearrange("p (b c) -> p b c", b=8)
                    sO = sin_t[:, 256:512].rearrange("p (b c) -> p b c", b=8)
                    t1 = stream.tile([P, 256], F32, name="t1")
                    t2 = stream.tile([P, 256], F32, name="t2")
                    t1v = t1[:].rearrange("p (b c) -> p b c", b=8)
                    t2v = t2[:].rearrange("p (b c) -> p b c", b=8)
                    qk_sb = stream.tile([P, 512], BF16, name="qk_sb")
                    qkv_out = qk_sb[:].rearrange("p (b c) -> p b c", b=8)
                    outE, outO = qkv_out[:, :, 0:32], qkv_out[:, :, 32:64]
                    nc.vector.tensor_tensor(out=t1v, in0=qE, in1=cE, op=ALU.mult)
                    nc.vector.tensor_tensor(out=t2v, in0=qO, in1=sE, op=ALU.mult)
                    nc.vector.tensor_tensor(out=outE, in0=t1v, in1=t2v,
                                            op=ALU.subtract)
                    t3 = stream.tile([P, 256], F32, name="t1")
                    t4 = stream.tile([P, 256], F32, name="t2")
                    t3v = t3[:].rearrange("p (b c) -> p b c", b=8)
                    t4v = t4[:].rearrange("p (b c) -> p b c", b=8)
                    nc.vector.tensor_tensor(out=t3v, in0=qO, in1=cO, op=ALU.mult)
                    nc.vector.tensor_tensor(out=t4v, in0=qE, in1=sO, op=ALU.mult)
                    nc.vector.tensor_tensor(out=outO, in0=t3v, in1=t4v, op=ALU.add)
                    # v: psum cols 512:768 -> v_sb
                    nc.vector.tensor_copy(v_sb[:, tok, :], v_ps[:])
                    # transpose rotated q|k into QKT
                    nc.sync.dma_start_transpose(QKT_sb[:, tok, :, :], qk_sb[:])

                # gates for this chunk: sigmoid(xn @ wg.T + b) via exp, then
                # scatter head h to partition 32*h of gates_sb (DVE operand
                # bases must be 32-aligned, DMA moves partitions freely)
                gates_ps = ps_ygs.tile([HL, 512], F32, name="gates_ps", tag="ygs")
                for kd in range(KD):
                    nc.tensor.matmul(gates_ps[:], wg_sb[:, kd, :],
                                     xnT[:, :, kd, :],
                                     start=(kd == 0), stop=(kd == KD - 1))
                ge = stream.tile([HL, 512], F32, name="ge")
                nc.scalar.activation(ge[:], gates_ps[:], AF.Exp, scale=-1.0,
                                     bias=bgn_sb[:])
                gp = stream.tile([HL, 512], F32, name="gp")
                nc.vector.tensor_scalar_add(gp[:], ge[:], 1.0)
                grec = stream.tile([HL, 512], F32, name="grec")
                nc.vector.reciprocal(grec[:], gp[:])
                nc.gpsimd.dma_start(
                    gates_sb[:, ci * 512:(ci + 1) * 512]
                    .rearrange("(a b) c -> a b c", b=32)[:, 0, :],
                    grec[:],
                )
                if dbg and ci == 0:
                    nc.gpsimd.dma_start(dbg["xnT"][:], xnT[:])

            # =========== Phase B: attention + out proj + RS ===========
            ydram = []
            rsout = []
            for ci in range(NQ):
                ydram.append(dramp.tile([QT, DIM], F32, name=f"ydram{ci}"))
                rsout.append(dramp.tile([P, DIM], F32, name=f"rsout{ci}"))

            if dbg:
                nc.gpsimd.dma_start(dbg["qkt"][:], QKT_sb[:])
                nc.gpsimd.dma_start(dbg["v"][:], v_sb[:])
                nc.gpsimd.dma_start(dbg["gates"][:], gates_sb[:, 0:512])

            nq_run = int(os.environ.get("KQUARTERS", NQ))
            for ci in range(nq_run):
                av01 = ps_qa1.tile([P, 512], F32, name="av01", tag="qa1")
                av23 = ps_qa2.tile([P, 512], F32, name="av23", tag="qa2")
                sums = ps_ygs.tile([97, 512], F32, name="sums", tag="ygs")
                islc = slice(ci * 512, (ci + 1) * 512)
                for jt in range(TT):
                    if (ci, jt) in pre_pts:
                        pts = pre_pts.pop((ci, jt))
                    else:
                        pts = emit_se(ci, jt)
                    emit_avs(jt, pts, av01, av23, sums)

                # prefetch the next quarter's first scores+exps so the ACT
                # queue has no gap across the quarter boundary
                if ci + 1 < nq_run:
                    for jt in range(4):
                        pre_pts[(ci + 1, jt)] = emit_se(ci + 1, jt)

                if dbg and ci == 0:
                    smd = tailp.tile([P, 512], F32, name="smd")
                    nc.vector.tensor_copy(smd[0:97, :], sums[0:97, :])
                    nc.gpsimd.dma_start(dbg["sums"][:], smd[:])
                    avd = tailp.tile([P, 512], F32, name="avd")
                    nc.vector.tensor_copy(avd[:], av01[:])
                    nc.gpsimd.dma_start(dbg["av"][:], avd[:])

                # normalize + gate -> outflatT. The per-query scale
                # c = gate/softmax_sum is broadcast across the 64 head dims
                # with a K=1 ones outer-product on the PE.
                oT = tailp.tile([P, 2, 512], BF16, name="oT")
                for hp in range(2):
                    c_ps = ps_s.tile([P, 512], F32, name="c_ps", tag="s")
                    for e in range(2):
                        h = 2 * hp + e
                        sr = tailp.tile([1, 512], F32, name="sr")
                        nc.vector.tensor_copy(sr[:], sums[h * 32:h * 32 + 1, :])
                        gr = tailp.tile([1, 512], F32, name="gr")
                        nc.vector.tensor_copy(gr[:], gates_sb[h * 32:h * 32 + 1, islc])
                        rc = tailp.tile([1, 512], F32, name="rc")
                        nc.vector.reciprocal(rc[:], sr[:])
                        cr = tailp.tile([1, 512], BF16, name="cr")
                        nc.vector.tensor_tensor(out=cr[:], in0=rc[:], in1=gr[:],
                                                op=ALU.mult)
                        nc.tensor.matmul(c_ps[e * 64:(e + 1) * 64, :],
                                         ones_row[:, :], cr[:],
                                         start=True, stop=True)
                    c_sb = tailp.tile([P, 512], F32, name="c_sb")
                    nc.vector.tensor_copy(c_sb[:], c_ps[:])
                    avt = av01 if hp == 0 else av23
                    nc.vector.tensor_tensor(out=oT[:, hp, :], in0=avt[:],
                                            in1=c_sb[:], op=ALU.mult)

                # output projection (partial over local heads)
                for tt in range(4):
                    for oc in range(2):
                        y_ps = ps_ygs.tile([P, 512], F32, name="y_ps", tag="ygs")
                        for kt in range(2):
                            nc.tensor.matmul(
                                y_ps[:],
                                oT[:, kt, tt * P:(tt + 1) * P],
                                wout_sb[:, kt, oc * 512:(oc + 1) * 512],
                                start=(kt == 0), stop=(kt == 1),
                            )
                        y_sb = tailp.tile([P, 512], F32, name="y_sb")
                        nc.vector.tensor_copy(y_sb[:], y_ps[:])
                        nc.sync.dma_start(
                            ydram[ci][tt * P:(tt + 1) * P,
                                      oc * 512:(oc + 1) * 512],
                            y_sb[:],
                        )

                if dbg and ci == 0:
                    nc.gpsimd.dma_start(dbg["oT"][:], oT[:])
                    nc.gpsimd.dma_start(dbg["y"][:], ydram[ci][:])

                if os.environ.get("KNOCOLL"):
                    nc.gpsimd.dma_start(out_ext[ci, :, :], ydram[ci][0:P, :])
                else:
                    nc.gpsimd.collective_compute(
                        "ReduceScatter", ALU.add,
                        replica_groups=REPLICA_GROUPS,
                        ins=[ydram[ci][:].opt()],
                        outs=[rsout[ci][:].opt()],
                    )
                    nc.sync.dma_start(out_ext[ci, :, :], rsout[ci][:])

    nc.compile()
    return nc


def _get_nc():
    global _nc_cache
    if _nc_cache is None:
        _nc_cache = _build()
    return _nc_cache


_PERM_EO = np.concatenate([np.arange(0, DH, 2), np.arange(1, DH, 2)])


def _shard(core, x, rotary_cos, rotary_sin, gamma, w_qkv, w_gates, b_gates, w_out):
    g, r = core // 4, core % 4
    heads = np.arange(4 * r, 4 * r + 4)
    wq = w_qkv[0 * DIM:1 * DIM] * gamma[None, :]
    wk = w_qkv[1 * DIM:2 * DIM] * gamma[None, :]
    wv = w_qkv[2 * DIM:3 * DIM]

    def qk_rows(w):
        # rows for local heads with even/odd permutation within each head
        idx = (heads[:, None] * DH + _PERM_EO[None, :]).reshape(-1)
        return w[idx]

    v_rows = wv[(heads[:, None] * DH + np.arange(DH)[None, :]).reshape(-1)]
    wqkv_t = np.concatenate([qk_rows(wq), qk_rows(wk), v_rows], axis=0).T
    wg_t = (w_gates[heads] * gamma[None, :]).T
    wout_t = w_out[:, heads[0] * DH:heads[0] * DH + HL * DH].T

    cos = rotary_cos[0, 0]  # (N, DH)
    sin = rotary_sin[0, 0]
    cosr = np.concatenate([np.tile(cos[:, 0::2], (1, 8)),
                           np.tile(cos[:, 1::2], (1, 8))], axis=1)
    sinr = np.concatenate([np.tile(sin[:, 0::2], (1, 8)),
                           np.tile(sin[:, 1::2], (1, 8))], axis=1)

    bf = ml_dtypes.bfloat16
    return {
        "x": np.ascontiguousarray(x[g], np.float32),
        "wqkv": np.ascontiguousarray(wqkv_t).astype(bf),
        "wg": np.ascontiguousarray(wg_t).astype(bf),
        "bgn": np.ascontiguousarray(-b_gates[heads].reshape(HL, 1), np.float32),
        "wout": np.ascontiguousarray(wout_t).astype(bf),
        "cosr": np.ascontiguousarray(cosr).astype(bf),
        "sinr": np.ascontiguousarray(sinr).astype(bf),
    }


def kernel(x, rotary_cos, rotary_sin, gamma, w_qkv, w_gates, b_gates, w_out):
    global _last_result
    args = [np.asarray(a, np.float32) for a in
            (x, rotary_cos, rotary_sin, gamma, w_qkv, w_gates, b_gates, w_out)]
    nc = _get_nc()
    in_maps = [_shard(c, *args) for c in range(CORES)]
    try:
        res = run_bass_kernel_spmd(
            nc, in_maps, core_ids=list(range(CORES)),
            trace=bool(os.environ.get("KTRACE")),
        )
    except ModuleNotFoundError:
        # profiler hook unavailable in this environment - run without trace
        res = run_bass_kernel_spmd(nc, in_maps, core_ids=list(range(CORES)))
    _last_result = res
    full = np.zeros((B, N, DIM), np.float32)
    for c in range(CORES):
        g, r = c // 4, c % 4
        o = np.asarray(res.results[c]["out"]).reshape(NQ, P, DIM)
        for q in range(NQ):
            full[g, q * 512 + r * P: q * 512 + (r + 1) * P, :] = o[q]
    return full



# revision 8
# speedup vs baseline: 1.4884x; 1.4884x over previous
"""Distributed Trainium2 kernel for nn_Attention_61332132987140.

Gated multi-head attention block: RMSNorm -> QKV proj -> RoPE -> softmax
attention -> sigmoid head gating -> output projection.

Sharding: 8 cores = 2 batch groups x 4-head groups (tensor parallel on
heads). Each core computes attention for its batch's full sequence over
its 4 heads and the partial output projection; a bf16 ReduceScatter over
each 4-core batch group sums the partials, leaving each core a disjoint
256-token slice per 1024-token half. The host reassembles the full
(2, 2048, 1024) output.

Key cost-model structure (vs the earlier baseline):
- activations arrive host-transposed (xT) so no on-device xn transposes
- attention-V matmuls run in the [query, dim] orientation: full 128
  output partitions per instruction (half the PE charge of [dim, query])
- softmax denominators are free-size-1 matmuls (ones column) instead of
  [1, 512]-output ones: ~130k PE cycles saved
- gates run in the [token, head] orientation: charge 4 instead of 512
- per-query gate/sum normalization is a per-partition tensor_scalar, no
  PE broadcast matmuls
- norm scale folds into per-tile scaled cos/sin tables (4x-mode DVE)
- all Ln activations grouped before any Exp: 2 act-table loads, not 9
- ReduceScatter in bf16 per half: 2 collectives instead of 4 fp32 ones
"""
import os
import sys

sys.path.insert(0, "/opt/trn_rl_repo")

import numpy as np
import ml_dtypes

import concourse.bass as bass
import concourse.mybir as mybir
import concourse.tile as tile
from concourse import bacc
from concourse.bass_utils import run_bass_kernel_spmd

F32 = mybir.dt.float32
BF16 = mybir.dt.bfloat16
AF = mybir.ActivationFunctionType
ALU = mybir.AluOpType

B, N, DIM = 2, 2048, 1024
HEADS, DH = 16, 64
HL = 4  # local heads per core
P = 128
TT = N // P  # 16 token tiles
KD = DIM // P  # 8 contraction tiles
NQ = 4  # quarters (512-query chunks)
CORES = 8
REPLICA_GROUPS = [[0, 1, 2, 3], [4, 5, 6, 7]]

_nc_cache = None
_last_result = None


def _build():
    nc = bacc.Bacc("TRN2", target_bir_lowering=False, debug=False, num_devices=CORES)

    xT_ext = nc.declare_dram_parameter("xT", [DIM, N], BF16, isOutput=False)
    xb_ext = nc.declare_dram_parameter("xb", [N, DIM], BF16, isOutput=False)
    wqkv_ext = nc.declare_dram_parameter("wqkv", [DIM, 772], BF16, isOutput=False)
    wout_ext = nc.declare_dram_parameter("wout", [2 * P, DIM], BF16, isOutput=False)
    cosP_ext = nc.declare_dram_parameter("cosP", [N, 512], BF16, isOutput=False)
    sinN_ext = nc.declare_dram_parameter("sinN", [N, 512], BF16, isOutput=False)
    bgn_ext = nc.declare_dram_parameter("bgn", [P, HL], F32, isOutput=False)
    out_ext = nc.declare_dram_parameter("out", [2, 2 * P, DIM], BF16, isOutput=True)

    with tile.TileContext(nc) as tc:
        with (
            tc.tile_pool(name="wpool", bufs=1) as wpool,
            tc.tile_pool(name="persist", bufs=1) as persist,
            tc.tile_pool(name="xbp", bufs=4) as xbp,
            tc.tile_pool(name="sqp", bufs=2) as sqp,
            tc.tile_pool(name="small", bufs=4) as small,
            tc.tile_pool(name="csp", bufs=4) as csp,
            tc.tile_pool(name="tup", bufs=6) as tup,
            tc.tile_pool(name="qksp", bufs=4) as qksp,
            tc.tile_pool(name="ptp", bufs=14) as ptp,
            tc.tile_pool(name="op", bufs=2) as op_pool,
            tc.tile_pool(name="otp", bufs=2) as otp,
            tc.tile_pool(name="ysq", bufs=2) as ysq,
            tc.tile_pool(name="ps_s", bufs=3, space="PSUM") as ps_s,
            tc.tile_pool(name="ps_qy", bufs=2, space="PSUM") as ps_qy,
            tc.tile_pool(name="ps_va", bufs=2, space="PSUM") as ps_va,
            tc.tile_pool(name="dram", bufs=1, space="DRAM") as dramp,
        ):
            # ---- constants / weights ----
            wqkv_sb = wpool.tile([P, KD, 772], BF16)
            nc.scalar.dma_start(wqkv_sb[:], wqkv_ext.rearrange("(k p) f -> p k f", p=P))
            wout_sb = wpool.tile([P, 2, DIM], BF16)
            nc.scalar.dma_start(wout_sb[:], wout_ext.rearrange("(k p) f -> p k f", p=P))
            bgn_sb = wpool.tile([P, HL], F32)
            nc.scalar.dma_start(bgn_sb[:], bgn_ext[:])
            cosP_sb = wpool.tile([P, TT, 512], BF16)
            nc.sync.dma_start(cosP_sb[:], cosP_ext.rearrange("(t p) f -> p t f", p=P))
            sinN_sb = wpool.tile([P, TT, 512], BF16)
            nc.sync.dma_start(sinN_sb[:], sinN_ext.rearrange("(t p) f -> p t f", p=P))
            zb = wpool.tile([P, 1], F32)
            nc.gpsimd.memset(zb[:], 0.0)
            lb32 = wpool.tile([P, 1], F32)
            nc.gpsimd.memset(lb32[:], float(np.log(32.0)))
            ones_mm = wpool.tile([P, 1], BF16)
            nc.gpsimd.memset(ones_mm[:], 1.0)
            ones1 = wpool.tile([1, P], BF16)
            nc.gpsimd.memset(ones1[:], 1.0)
            zrow = wpool.tile([1, 512], BF16)
            nc.gpsimd.memset(zrow[:], 0.0)

            # ---- persistent activations ----
            xT_sb = persist.tile([P, KD, N], BF16)
            for c in range(4):
                nc.gpsimd.dma_start(
                    xT_sb[:, :, c * 512:(c + 1) * 512],
                    xT_ext.rearrange("(k p) t -> p k t", p=P)[:, :, c * 512:(c + 1) * 512],
                )
            # qkT blocks: 0=q(h0,h1) 1=q(h2,h3) 2=k(h0,h1) 3=k(h2,h3); rows=dh
            qkT_sb = persist.tile([P, 4, N], BF16)
            v_sb = persist.tile([P, TT, 256], BF16)
            gates_sb = persist.tile([P, TT, HL], F32)
            ss_all = persist.tile([P, TT], F32)
            s_all = persist.tile([P, TT], F32)

            # ---- norm: sumsq per token tile, then batched Ln+Exp ----
            for tt in range(TT):
                xb_t = xbp.tile([P, DIM], BF16, name="xb_t")
                nc.gpsimd.dma_start(xb_t[:], xb_ext[tt * P:(tt + 1) * P, :])
                scr = sqp.tile([P, DIM], BF16, name="scr")
                nc.vector.scalar_tensor_tensor(
                    out=scr[:], in0=xb_t[:], scalar=1.0, in1=xb_t[:],
                    op0=ALU.mult, op1=ALU.mult,
                    accum_out=ss_all[:, tt:tt + 1],
                )
            # s = 32 * ss^-0.5 = exp(-0.5*ln(ss) + ln32); one Ln + one Exp inst
            ln16 = small.tile([P, TT], F32, name="ln16")
            nc.scalar.activation(ln16[:], ss_all[:], AF.Ln, bias=zb[:])
            nc.scalar.activation(s_all[:], ln16[:], AF.Exp, scale=-0.5, bias=lb32[:])

            # ---- phase A per token tile: QKV + RoPE + transposes + gates ----
            for tt in range(TT):
                s_ap = s_all[:, tt:tt + 1]
                qk_ps = ps_qy.tile([P, 512], F32, name="qk_ps", tag="qy")
                vg_ps = ps_va.tile([P, 260], F32, name="vg_ps", tag="va")
                for kd in range(KD):
                    lhsT = xT_sb[:, kd, tt * P:(tt + 1) * P]
                    nc.tensor.matmul(qk_ps[:], lhsT, wqkv_sb[:, kd, 0:512],
                                     start=(kd == 0), stop=(kd == KD - 1))
                    nc.tensor.matmul(vg_ps[:], lhsT, wqkv_sb[:, kd, 512:772],
                                     start=(kd == 0), stop=(kd == KD - 1))

                # scaled rope tables: cos' = s*cos, sin' = s*sin(+-)
                cosS = csp.tile([P, 512], BF16, name="cosS")
                nc.vector.tensor_scalar(out=cosS[:], in0=cosP_sb[:, tt, :],
                                        scalar1=s_ap, scalar2=None, op0=ALU.mult)
                sinS = csp.tile([P, 512], BF16, name="sinS")
                nc.vector.tensor_scalar(out=sinS[:], in0=sinN_sb[:, tt, :],
                                        scalar1=s_ap, scalar2=None, op0=ALU.mult)

                # rope: qk' = qk*cos' + swap_eo(qk)*sin'  (u built in 2 halves)
                t_sb = tup.tile([P, 512], BF16, name="t_sb")
                nc.vector.tensor_tensor(out=t_sb[:], in0=qk_ps[:], in1=cosS[:],
                                        op=ALU.mult)
                u_sb = tup.tile([P, 512], BF16, name="u_sb")
                qkv8 = qk_ps[:].rearrange("p (b c) -> p b c", b=8)
                u8 = u_sb[:].rearrange("p (b c) -> p b c", b=8)
                sin8 = sinS[:].rearrange("p (b c) -> p b c", b=8)
                nc.vector.tensor_tensor(out=u8[:, :, 0:32], in0=qkv8[:, :, 32:64],
                                        in1=sin8[:, :, 0:32], op=ALU.mult)
                nc.vector.tensor_tensor(out=u8[:, :, 32:64], in0=qkv8[:, :, 0:32],
                                        in1=sin8[:, :, 32:64], op=ALU.mult)
                qk_sb = qksp.tile([P, 512], BF16, name="qk_sb")
                nc.vector.tensor_tensor(out=qk_sb[:], in0=t_sb[:], in1=u_sb[:],
                                        op=ALU.add)
                nc.sync.dma_start_transpose(qkT_sb[:, :, tt * P:(tt + 1) * P], qk_sb[:])

                # v with norm scale
                nc.vector.tensor_scalar(out=v_sb[:, tt, :], in0=vg_ps[:, 0:256],
                                        scalar1=s_ap, scalar2=None, op0=ALU.mult)

                # gates: sigmoid(s*z + b) via exp
                zt = small.tile([P, HL], BF16, name="zt")
                nc.vector.scalar_tensor_tensor(out=zt[:], in0=vg_ps[:, 256:260],
                                               scalar=s_ap, in1=bgn_sb[:],
                                               op0=ALU.mult, op1=ALU.add)
                ge = small.tile([P, HL], F32, name="ge")
                nc.scalar.activation(ge[:], zt[:], AF.Exp, scale=-1.0, bias=zb[:])
                gp = small.tile([P, HL], F32, name="gp")
                nc.vector.tensor_scalar_add(gp[:], ge[:], 1.0)
                nc.vector.reciprocal(gates_sb[:, tt, :], gp[:])

            # ---- phase B: attention + out proj + RS ----
            ydram = [dramp.tile([2 * 512, DIM], BF16, name=f"ydram{h}")
                     for h in range(2)]
            rsout = [dramp.tile([2 * P, DIM], BF16, name=f"rsout{h}")
                     for h in range(2)]

            # Schraudolph exp-to-bf16 constants: bits = s*0.125*log2e*128 + B
            SCH_A = float(0.125 * np.log2(np.e) * 128.0)
            SCH_B = float(16256.5 - 2.75)

            def emit_scores(ci, m):
                """scores+exp for jt pair (2m, 2m+1), all 4 heads.

                exp runs on ACT except jt%4==3 which uses the DVE
                Schraudolph bits trick (tensor_scalar into a uint16 view).
                """
                pts = []
                for h in range(4):
                    pt = ptp.tile([P, 2, 512], BF16, name="pt")
                    for par in range(2):
                        jt = 2 * m + par
                        hh = 64 * (h % 2)
                        s_t = ps_s.tile([P, 512], F32, name="s_t", tag="s")
                        nc.tensor.matmul(
                            s_t[:],
                            qkT_sb[hh:hh + 64, 2 + h // 2, jt * P:(jt + 1) * P],
                            qkT_sb[hh:hh + 64, h // 2, ci * 512:(ci + 1) * 512],
                            start=True, stop=True,
                        )
                        if jt % 4 == 3:
                            nc.vector.tensor_scalar(
                                out=pt[:, par, :].bitcast(mybir.dt.uint16),
                                in0=s_t[:], scalar1=SCH_A, scalar2=SCH_B,
                                op0=ALU.mult, op1=ALU.add)
                        else:
                            nc.scalar.activation(pt[:, par, :], s_t[:], AF.Exp,
                                                 scale=0.125, bias=zb[:])
                    pts.append(pt)
                return pts

            def emit_av(m, pts, av_t, sums_t):
                for h in range(4):
                    for qt in range(4):
                        for par in range(2):
                            jt = 2 * m + par
                            nc.tensor.matmul(
                                av_t[qt][:, h * DH:(h + 1) * DH],
                                pts[h][:, par, qt * P:(qt + 1) * P],
                                v_sb[:, jt, h * DH:(h + 1) * DH],
                                start=False, stop=(m == TT // 2 - 1 and par == 1),
                                skip_group_check=True,
                            )
                            nc.tensor.matmul(
                                sums_t[:, (h * 4 + qt):(h * 4 + qt) + 1],
                                pts[h][:, par, qt * P:(qt + 1) * P],
                                ones_mm[:, 0:1],
                                start=False, stop=(m == TT // 2 - 1 and par == 1),
                                skip_group_check=True,
                            )

            def emit_tail(ci, av_pair, sums_t, ydst):
                """normalize, gate, transpose, out-projection for quarter ci."""
                av_t = [av_pair[qt // 2][:, qt % 2, :] for qt in range(4)]
                oT = otp.tile([P, 2, 512], BF16, name="oT")
                for qt in range(4):
                    rec = small.tile([P, HL], F32, name="rec")
                    nc.vector.reciprocal(
                        rec[:],
                        sums_t[:, 0:16].rearrange("p (h q) -> p q h", q=4)[:, qt, :])
                    scl = small.tile([P, HL], F32, name="scl")
                    nc.vector.tensor_tensor(out=scl[:], in0=rec[:],
                                            in1=gates_sb[:, ci * 4 + qt, :],
                                            op=ALU.mult)
                    o_sb = op_pool.tile([P, 256], BF16, name="o_sb")
                    for h in range(4):
                        nc.vector.tensor_scalar(
                            out=o_sb[:, h * DH:(h + 1) * DH],
                            in0=av_t[qt][:, h * DH:(h + 1) * DH],
                            scalar1=scl[:, h:h + 1], scalar2=None, op0=ALU.mult)
                    nc.sync.dma_start_transpose(oT[:, :, qt * P:(qt + 1) * P],
                                                o_sb[:])
                y_sbq = ysq.tile([P, 4, DIM], BF16, name="y_sbq")
                for qt in range(4):
                    for oh in range(2):
                        y_ps = ps_qy.tile([P, 512], F32, name="y_ps", tag="qy")
                        for kh in range(2):
                            nc.tensor.matmul(
                                y_ps[:],
                                oT[:, kh, qt * P:(qt + 1) * P],
                                wout_sb[:, kh, oh * 512:(oh + 1) * 512],
                                start=(kh == 0), stop=(kh == 1),
                            )
                        if oh == 0:
                            nc.scalar.activation(
                                y_sbq[:, qt, oh * 512:(oh + 1) * 512], y_ps[:],
                                AF.Copy, bias=0.0)
                        else:
                            nc.vector.tensor_copy(
                                y_sbq[:, qt, oh * 512:(oh + 1) * 512], y_ps[:])
                nc.gpsimd.dma_start(ydst, y_sbq[:])

            pending_tail = None
            for ci in range(NQ):
                av_pair = [ps_va.tile([P, 2, 256], F32, name=f"av{i}", tag="va")
                           for i in range(2)]
                av_t = [av_pair[qt // 2][:, qt % 2, :] for qt in range(4)]
                sums_t = ps_s.tile([P, 16], F32, name="sums_t", tag="sums", bufs=1)
                # zero the accumulator banks via K=1 zero matmuls so the
                # col-disjoint accumulation groups can all run start=False
                # (a start=True would mark the whole bank pending-zero and
                # wipe sibling groups)
                for i in range(2):
                    nc.tensor.matmul(av_pair[i][:], ones1[:], zrow[:],
                                     start=True, stop=True)
                nc.tensor.matmul(sums_t[:], ones1[:], zrow[:, 0:16],
                                 start=True, stop=True)
                for m in range(TT // 2):
                    pts = emit_scores(ci, m)
                    emit_av(m, pts, av_t, sums_t)
                    # defer previous quarter's tail past the first jt pairs so
                    # the PE keeps streaming scores while DVE/DMA run the tail
                    if m == 1 and pending_tail is not None:
                        emit_tail(*pending_tail)
                        pending_tail = None
                hf = ci // 2
                ydst = ydram[hf][(ci % 2) * 512:(ci % 2) * 512 + 512, :] \
                    .rearrange("(q p) o -> p q o", p=P)
                pending_tail = (ci, av_pair, sums_t, ydst)
                if ci % 2 == 1:
                    emit_tail(*pending_tail)
                    pending_tail = None
                    nc.gpsimd.collective_compute(
                        "ReduceScatter", ALU.add,
                        replica_groups=REPLICA_GROUPS,
                        ins=[ydram[hf][:].opt()],
                        outs=[rsout[hf][:].opt()],
                    )
                    nc.sync.dma_start(out_ext[hf, :, :], rsout[hf][:])

    nc.compile()
    return nc


def _get_nc():
    global _nc_cache
    if _nc_cache is None:
        _nc_cache = _build()
    return _nc_cache


_PERM_EO = np.concatenate([np.arange(0, DH, 2), np.arange(1, DH, 2)])


def _shard(core, x, rotary_cos, rotary_sin, gamma, w_qkv, w_gates, b_gates, w_out):
    g, r = core // 4, core % 4
    heads = np.arange(4 * r, 4 * r + 4)
    wq = w_qkv[0 * DIM:1 * DIM] * gamma[None, :]
    wk = w_qkv[1 * DIM:2 * DIM] * gamma[None, :]
    wv = w_qkv[2 * DIM:3 * DIM]

    def qk_rows(w):
        idx = (heads[:, None] * DH + _PERM_EO[None, :]).reshape(-1)
        return w[idx]

    v_rows = wv[(heads[:, None] * DH + np.arange(DH)[None, :]).reshape(-1)]
    wg_rows = w_gates[heads] * gamma[None, :]
    wqkv_t = np.concatenate([qk_rows(wq), qk_rows(wk), v_rows, wg_rows],
                            axis=0).T
    wout_t = w_out[:, heads[0] * DH:heads[0] * DH + HL * DH].T

    cos = rotary_cos[0, 0]  # (N, DH)
    sin = rotary_sin[0, 0]
    cosP = np.tile(np.concatenate([cos[:, 0::2], cos[:, 1::2]], axis=1), (1, 8))
    sinN = np.tile(np.concatenate([-sin[:, 0::2], sin[:, 1::2]], axis=1), (1, 8))

    bf = ml_dtypes.bfloat16
    return {
        "xT": np.ascontiguousarray(x[g].T).astype(bf),
        "xb": np.ascontiguousarray(x[g]).astype(bf),
        "wqkv": np.ascontiguousarray(wqkv_t).astype(bf),
        "wout": np.ascontiguousarray(wout_t).astype(bf),
        "cosP": np.ascontiguousarray(cosP).astype(bf),
        "sinN": np.ascontiguousarray(sinN).astype(bf),
        "bgn": np.tile(b_gates[heads][None, :], (P, 1)).astype(np.float32),
    }


def kernel(x, rotary_cos, rotary_sin, gamma, w_qkv, w_gates, b_gates, w_out):
    global _last_result
    args = [np.asarray(a, np.float32) for a in
            (x, rotary_cos, rotary_sin, gamma, w_qkv, w_gates, b_gates, w_out)]
    nc = _get_nc()
    in_maps = [_shard(c, *args) for c in range(CORES)]
    try:
        res = run_bass_kernel_spmd(
            nc, in_maps, core_ids=list(range(CORES)),
            trace=bool(os.environ.get("KTRACE")),
        )
    except ModuleNotFoundError:
        res = run_bass_kernel_spmd(nc, in_maps, core_ids=list(range(CORES)))
    _last_result = res
    full = np.zeros((B, N, DIM), np.float32)
    for c in range(CORES):
        g, r = c // 4, c % 4
        o = np.asarray(res.results[c]["out"]).astype(np.float32)
        for hf in range(2):
            full[g, hf * 1024 + r * 256: hf * 1024 + (r + 1) * 256, :] = o[hf]
    return full


# revision 12
# speedup vs baseline: 1.5653x; 1.0517x over previous
"""Distributed Trainium2 kernel for nn_Attention_61332132987140.

Gated multi-head attention block: RMSNorm -> QKV proj -> RoPE -> softmax
attention -> sigmoid head gating -> output projection.

Sharding: 8 cores = 2 batch groups x 4-head groups (tensor parallel on
heads). Each core computes attention for its batch's full sequence over
its 4 heads and the partial output projection; a bf16 ReduceScatter over
each 4-core batch group sums the partials, leaving each core a disjoint
256-token slice per 1024-token half. The host reassembles the full
(2, 2048, 1024) output.

Key cost-model structure (vs the earlier baseline):
- activations arrive host-transposed (xT) so no on-device xn transposes
- attention-V matmuls run in the [query, dim] orientation: full 128
  output partitions per instruction (half the PE charge of [dim, query])
- softmax denominators are free-size-1 matmuls (ones column) instead of
  [1, 512]-output ones: ~130k PE cycles saved
- gates run in the [token, head] orientation: charge 4 instead of 512
- per-query gate/sum normalization is a per-partition tensor_scalar, no
  PE broadcast matmuls
- norm scale folds into per-tile scaled cos/sin tables (4x-mode DVE)
- all Ln activations grouped before any Exp: 2 act-table loads, not 9
- ReduceScatter in bf16 per half: 2 collectives instead of 4 fp32 ones
"""
import os
import sys

sys.path.insert(0, "/opt/trn_rl_repo")

import numpy as np
import ml_dtypes

import concourse.bass as bass
import concourse.mybir as mybir
import concourse.tile as tile
from concourse import bacc
from concourse.bass_utils import run_bass_kernel_spmd

F32 = mybir.dt.float32
BF16 = mybir.dt.bfloat16
I32 = mybir.dt.int32
U16 = mybir.dt.uint16
AF = mybir.ActivationFunctionType
ALU = mybir.AluOpType

B, N, DIM = 2, 2048, 1024
HEADS, DH = 16, 64
HL = 4  # local heads per core
P = 128
TT = N // P  # 16 token tiles
KD = DIM // P  # 8 contraction tiles
NQ = 4  # quarters (512-query chunks)
CORES = 8
REPLICA_GROUPS = [[0, 1, 2, 3], [4, 5, 6, 7]]

_nc_cache = None
_last_result = None


def _build():
    nc = bacc.Bacc("TRN2", target_bir_lowering=False, debug=False, num_devices=CORES)

    xT_ext = nc.declare_dram_parameter("xT", [DIM, N], BF16, isOutput=False)
    xb_ext = nc.declare_dram_parameter("xb", [N, DIM], BF16, isOutput=False)
    wqkv_ext = nc.declare_dram_parameter("wqkv", [DIM, 772], BF16, isOutput=False)
    wout_ext = nc.declare_dram_parameter("wout", [2 * P, DIM], BF16, isOutput=False)
    cosP_ext = nc.declare_dram_parameter("cosP", [N, 512], BF16, isOutput=False)
    sinN_ext = nc.declare_dram_parameter("sinN", [N, 512], BF16, isOutput=False)
    bgn_ext = nc.declare_dram_parameter("bgn", [P, HL], F32, isOutput=False)
    out_ext = nc.declare_dram_parameter("out", [2, 2 * P, DIM], BF16, isOutput=True)

    with tile.TileContext(nc) as tc:
        with (
            tc.tile_pool(name="wpool", bufs=1) as wpool,
            tc.tile_pool(name="persist", bufs=1) as persist,
            tc.tile_pool(name="xbp", bufs=4) as xbp,
            tc.tile_pool(name="sqp", bufs=2) as sqp,
            tc.tile_pool(name="small", bufs=4) as small,
            tc.tile_pool(name="csp", bufs=4) as csp,
            tc.tile_pool(name="tup", bufs=6) as tup,
            tc.tile_pool(name="qksp", bufs=4) as qksp,
            tc.tile_pool(name="ptp", bufs=14) as ptp,
            tc.tile_pool(name="op", bufs=2) as op_pool,
            tc.tile_pool(name="otp", bufs=2) as otp,
            tc.tile_pool(name="ysq", bufs=2) as ysq,
            tc.tile_pool(name="ps_s", bufs=3, space="PSUM") as ps_s,
            tc.tile_pool(name="ps_qy", bufs=2, space="PSUM") as ps_qy,
            tc.tile_pool(name="ps_va", bufs=2, space="PSUM") as ps_va,
            tc.tile_pool(name="dram", bufs=1, space="DRAM") as dramp,
        ):
            # ---- constants / weights ----
            wqkv_sb = wpool.tile([P, KD, 772], BF16)
            nc.scalar.dma_start(wqkv_sb[:], wqkv_ext.rearrange("(k p) f -> p k f", p=P))
            wout_sb = wpool.tile([P, 2, DIM], BF16)
            nc.scalar.dma_start(wout_sb[:], wout_ext.rearrange("(k p) f -> p k f", p=P))
            bgn_sb = wpool.tile([P, HL], F32)
            nc.scalar.dma_start(bgn_sb[:], bgn_ext[:])
            cosP_sb = wpool.tile([P, TT, 512], BF16)
            nc.sync.dma_start(cosP_sb[:], cosP_ext.rearrange("(t p) f -> p t f", p=P))
            sinN_sb = wpool.tile([P, TT, 512], BF16)
            nc.sync.dma_start(sinN_sb[:], sinN_ext.rearrange("(t p) f -> p t f", p=P))
            zb = wpool.tile([P, 1], F32)
            nc.gpsimd.memset(zb[:], 0.0)
            ones_mm = wpool.tile([P, 1], BF16)
            nc.gpsimd.memset(ones_mm[:], 1.0)
            ones1 = wpool.tile([1, P], BF16)
            nc.gpsimd.memset(ones1[:], 1.0)
            zrow = wpool.tile([1, 512], BF16)
            nc.gpsimd.memset(zrow[:], 0.0)

            # ---- persistent activations ----
            xT_sb = persist.tile([P, KD, N], BF16)
            for c in range(4):
                nc.gpsimd.dma_start(
                    xT_sb[:, :, c * 512:(c + 1) * 512],
                    xT_ext.rearrange("(k p) t -> p k t", p=P)[:, :, c * 512:(c + 1) * 512],
                )
            # qkT blocks: 0=q(h0,h1) 1=q(h2,h3) 2=k(h0,h1) 3=k(h2,h3); rows=dh
            qkT_sb = persist.tile([P, 4, N], BF16)
            v_sb = persist.tile([P, TT, 256], BF16)
            gates_sb = persist.tile([P, TT, HL], F32)
            ss_all = persist.tile([P, TT], F32)
            s_all = persist.tile([P, TT], F32)

            def emit_norm_group(g):
                """sumsq (STT w/ accum) + quake rsqrt for token tiles 4g..4g+3.

                s = 32*ss^-0.5 computed entirely on DVE/Pool (no act tables):
                quake seed on ss/1024 folded magic + 2 Newton iterations.
                """
                for tt in range(4 * g, 4 * g + 4):
                    xb_t = xbp.tile([P, DIM], BF16, name="xb_t")
                    nc.gpsimd.dma_start(xb_t[:], xb_ext[tt * P:(tt + 1) * P, :])
                    scr = sqp.tile([P, DIM], BF16, name="scr")
                    nc.vector.scalar_tensor_tensor(
                        out=scr[:], in0=xb_t[:], scalar=1.0, in1=xb_t[:],
                        op0=ALU.mult, op1=ALU.mult,
                        accum_out=ss_all[:, tt:tt + 1],
                    )
                gs = slice(4 * g, 4 * g + 4)
                xp = small.tile([P, 4], F32, name="xp")
                nc.vector.tensor_scalar(out=xp[:], in0=ss_all[:, gs],
                                        scalar1=1.0 / 1024.0, scalar2=None,
                                        op0=ALU.mult)
                jt_ = small.tile([P, 4], I32, name="jt_")
                nc.vector.tensor_scalar(out=jt_[:], in0=xp[:].bitcast(I32),
                                        scalar1=1, scalar2=None,
                                        op0=ALU.logical_shift_right)
                y0b = small.tile([P, 4], I32, name="y0b")
                nc.vector.tensor_scalar(out=y0b[:], in0=jt_[:], scalar1=-1,
                                        scalar2=0x5f3759df, op0=ALU.mult,
                                        op1=ALU.add)
                cur = y0b[:].bitcast(F32)
                for it in range(2):
                    h_ = small.tile([P, 4], F32, name="h_")
                    nc.vector.tensor_tensor(out=h_[:], in0=cur, in1=cur,
                                            op=ALU.mult)
                    hx = small.tile([P, 4], F32, name="hx")
                    nc.vector.tensor_tensor(out=hx[:], in0=h_[:], in1=xp[:],
                                            op=ALU.mult)
                    w_ = small.tile([P, 4], F32, name="w_")
                    nc.vector.tensor_scalar(out=w_[:], in0=hx[:], scalar1=-0.5,
                                            scalar2=1.5, op0=ALU.mult,
                                            op1=ALU.add)
                    dst = s_all[:, gs] if it == 1 else \
                        small.tile([P, 4], F32, name="nx")[:]
                    nc.vector.tensor_tensor(out=dst, in0=cur, in1=w_[:],
                                            op=ALU.mult)
                    cur = dst

            # ---- phase A per token tile: QKV + RoPE + transposes + gates ----
            for tt in range(TT):
                if tt % 4 == 0:
                    emit_norm_group(tt // 4)
                s_ap = s_all[:, tt:tt + 1]
                qk_ps = ps_qy.tile([P, 512], F32, name="qk_ps", tag="qy")
                vg_ps = ps_va.tile([P, 260], F32, name="vg_ps", tag="va")
                for kd in range(KD):
                    lhsT = xT_sb[:, kd, tt * P:(tt + 1) * P]
                    nc.tensor.matmul(qk_ps[:], lhsT, wqkv_sb[:, kd, 0:512],
                                     start=(kd == 0), stop=(kd == KD - 1))
                    nc.tensor.matmul(vg_ps[:], lhsT, wqkv_sb[:, kd, 512:772],
                                     start=(kd == 0), stop=(kd == KD - 1))

                # scaled rope tables: cos' = s*cos, sin' = s*sin(+-)
                cosS = csp.tile([P, 512], BF16, name="cosS")
                nc.vector.tensor_scalar(out=cosS[:], in0=cosP_sb[:, tt, :],
                                        scalar1=s_ap, scalar2=None, op0=ALU.mult)
                sinS = csp.tile([P, 512], BF16, name="sinS")
                nc.vector.tensor_scalar(out=sinS[:], in0=sinN_sb[:, tt, :],
                                        scalar1=s_ap, scalar2=None, op0=ALU.mult)

                # rope: qk' = qk*cos' + swap_eo(qk)*sin'  (u built in 2 halves)
                t_sb = tup.tile([P, 512], BF16, name="t_sb")
                nc.vector.tensor_tensor(out=t_sb[:], in0=qk_ps[:], in1=cosS[:],
                                        op=ALU.mult)
                u_sb = tup.tile([P, 512], BF16, name="u_sb")
                qkv8 = qk_ps[:].rearrange("p (b c) -> p b c", b=8)
                u8 = u_sb[:].rearrange("p (b c) -> p b c", b=8)
                sin8 = sinS[:].rearrange("p (b c) -> p b c", b=8)
                nc.vector.tensor_tensor(out=u8[:, :, 0:32], in0=qkv8[:, :, 32:64],
                                        in1=sin8[:, :, 0:32], op=ALU.mult)
                nc.vector.tensor_tensor(out=u8[:, :, 32:64], in0=qkv8[:, :, 0:32],
                                        in1=sin8[:, :, 32:64], op=ALU.mult)
                qk_sb = qksp.tile([P, 512], BF16, name="qk_sb")
                nc.gpsimd.tensor_tensor(out=qk_sb[:], in0=t_sb[:], in1=u_sb[:],
                                        op=ALU.add)
                nc.sync.dma_start_transpose(qkT_sb[:, :, tt * P:(tt + 1) * P], qk_sb[:])

                # v with norm scale
                nc.vector.tensor_scalar(out=v_sb[:, tt, :], in0=vg_ps[:, 0:256],
                                        scalar1=s_ap, scalar2=None, op0=ALU.mult)

                # gates: sigmoid(s*z + b) via exp
                zt = small.tile([P, HL], BF16, name="zt")
                nc.vector.scalar_tensor_tensor(out=zt[:], in0=vg_ps[:, 256:260],
                                               scalar=s_ap, in1=bgn_sb[:],
                                               op0=ALU.mult, op1=ALU.add)
                ge = small.tile([P, HL], F32, name="ge")
                nc.scalar.activation(ge[:], zt[:], AF.Exp, scale=-1.0, bias=zb[:])
                gp = small.tile([P, HL], F32, name="gp")
                nc.vector.tensor_scalar_add(gp[:], ge[:], 1.0)
                nc.vector.reciprocal(gates_sb[:, tt, :], gp[:])

            # ---- phase B: attention + out proj + RS ----
            ydram = [dramp.tile([2 * 512, DIM], BF16, name=f"ydram{h}")
                     for h in range(2)]
            rsout = [dramp.tile([2 * P, DIM], BF16, name=f"rsout{h}")
                     for h in range(2)]

            # Schraudolph exp-to-bf16 constants: bits = s*0.125*log2e*128 + B
            SCH_A = float(0.125 * np.log2(np.e) * 128.0)
            SCH_B = float(16256.5 - 5.5)

            def emit_scores(ci, m):
                """scores+exp for jt pair (2m, 2m+1), all 4 heads.

                exp runs on ACT except jt%4==3 which uses the DVE
                Schraudolph bits trick (tensor_scalar into a uint16 view).
                """
                pts = []
                for h in range(4):
                    pt = ptp.tile([P, 2, 512], BF16, name="pt")
                    for par in range(2):
                        jt = 2 * m + par
                        hh = 64 * (h % 2)
                        s_t = ps_s.tile([P, 512], F32, name="s_t", tag="s")
                        nc.tensor.matmul(
                            s_t[:],
                            qkT_sb[hh:hh + 64, 2 + h // 2, jt * P:(jt + 1) * P],
                            qkT_sb[hh:hh + 64, h // 2, ci * 512:(ci + 1) * 512],
                            start=True, stop=True,
                        )
                        if jt % 2 == 1:
                            nc.vector.tensor_scalar(
                                out=pt[:, par, :].bitcast(U16),
                                in0=s_t[:], scalar1=SCH_A, scalar2=SCH_B,
                                op0=ALU.mult, op1=ALU.add)
                        else:
                            nc.scalar.activation(pt[:, par, :], s_t[:], AF.Exp,
                                                 scale=0.125, bias=zb[:])
                    pts.append(pt)
                return pts

            def emit_av(m, pts, av_t, sums_t):
                for h in range(4):
                    for qt in range(4):
                        for par in range(2):
                            jt = 2 * m + par
                            nc.tensor.matmul(
                                av_t[qt][:, h * DH:(h + 1) * DH],
                                pts[h][:, par, qt * P:(qt + 1) * P],
                                v_sb[:, jt, h * DH:(h + 1) * DH],
                                start=False, stop=(m == TT // 2 - 1 and par == 1),
                                skip_group_check=True,
                            )
                            nc.tensor.matmul(
                                sums_t[:, (h * 4 + qt):(h * 4 + qt) + 1],
                                pts[h][:, par, qt * P:(qt + 1) * P],
                                ones_mm[:, 0:1],
                                start=False, stop=(m == TT // 2 - 1 and par == 1),
                                skip_group_check=True,
                            )

            def emit_tail(ci, av_pair, sums_t, ydst):
                """normalize, gate, transpose, out-projection for quarter ci."""
                av_t = [av_pair[qt // 2][:, qt % 2, :] for qt in range(4)]
                oT = otp.tile([P, 2, 512], BF16, name="oT")
                for qt in range(4):
                    rec = small.tile([P, HL], F32, name="rec")
                    nc.vector.reciprocal(
                        rec[:],
                        sums_t[:, 0:16].rearrange("p (h q) -> p q h", q=4)[:, qt, :])
                    scl = small.tile([P, HL], F32, name="scl")
                    nc.vector.tensor_tensor(out=scl[:], in0=rec[:],
                                            in1=gates_sb[:, ci * 4 + qt, :],
                                            op=ALU.mult)
                    o_sb = op_pool.tile([P, 256], BF16, name="o_sb")
                    for h in range(4):
                        nc.vector.tensor_scalar(
                            out=o_sb[:, h * DH:(h + 1) * DH],
                            in0=av_t[qt][:, h * DH:(h + 1) * DH],
                            scalar1=scl[:, h:h + 1], scalar2=None, op0=ALU.mult)
                    nc.sync.dma_start_transpose(oT[:, :, qt * P:(qt + 1) * P],
                                                o_sb[:])
                y_sbq = ysq.tile([P, 4, DIM], BF16, name="y_sbq")
                for qt in range(4):
                    for oh in range(2):
                        y_ps = ps_qy.tile([P, 512], F32, name="y_ps", tag="qy")
                        for kh in range(2):
                            nc.tensor.matmul(
                                y_ps[:],
                                oT[:, kh, qt * P:(qt + 1) * P],
                                wout_sb[:, kh, oh * 512:(oh + 1) * 512],
                                start=(kh == 0), stop=(kh == 1),
                            )
                        if oh == 0:
                            nc.scalar.activation(
                                y_sbq[:, qt, oh * 512:(oh + 1) * 512], y_ps[:],
                                AF.Copy, bias=0.0)
                        else:
                            nc.vector.tensor_copy(
                                y_sbq[:, qt, oh * 512:(oh + 1) * 512], y_ps[:])
                nc.gpsimd.dma_start(ydst, y_sbq[:])

            pending_tail = None
            for ci in range(NQ):
                av_pair = [ps_va.tile([P, 2, 256], F32, name=f"av{i}", tag="va")
                           for i in range(2)]
                av_t = [av_pair[qt // 2][:, qt % 2, :] for qt in range(4)]
                sums_t = ps_s.tile([P, 16], F32, name="sums_t", tag="sums", bufs=1)
                # zero the accumulator banks via K=1 zero matmuls so the
                # col-disjoint accumulation groups can all run start=False
                # (a start=True would mark the whole bank pending-zero and
                # wipe sibling groups)
                for i in range(2):
                    nc.tensor.matmul(av_pair[i][:], ones1[:], zrow[:],
                                     start=True, stop=True)
                nc.tensor.matmul(sums_t[:], ones1[:], zrow[:, 0:16],
                                 start=True, stop=True)
                for m in range(TT // 2):
                    pts = emit_scores(ci, m)
                    emit_av(m, pts, av_t, sums_t)
                    # defer previous quarter's tail past the first jt pairs so
                    # the PE keeps streaming scores while DVE/DMA run the tail
                    if m == 1 and pending_tail is not None:
                        emit_tail(*pending_tail)
                        pending_tail = None
                hf = ci // 2
                ydst = ydram[hf][(ci % 2) * 512:(ci % 2) * 512 + 512, :] \
                    .rearrange("(q p) o -> p q o", p=P)
                pending_tail = (ci, av_pair, sums_t, ydst)
                if ci % 2 == 1:
                    emit_tail(*pending_tail)
                    pending_tail = None
                    nc.gpsimd.collective_compute(
                        "ReduceScatter", ALU.add,
                        replica_groups=REPLICA_GROUPS,
                        ins=[ydram[hf][:].opt()],
                        outs=[rsout[hf][:].opt()],
                    )
                    nc.gpsimd.dma_start(out_ext[hf, :, :], rsout[hf][:])

    nc.compile()
    return nc


def _get_nc():
    global _nc_cache
    if _nc_cache is None:
        _nc_cache = _build()
    return _nc_cache


_PERM_EO = np.concatenate([np.arange(0, DH, 2), np.arange(1, DH, 2)])


def _shard(core, x, rotary_cos, rotary_sin, gamma, w_qkv, w_gates, b_gates, w_out):
    g, r = core // 4, core % 4
    heads = np.arange(4 * r, 4 * r + 4)
    wq = w_qkv[0 * DIM:1 * DIM] * gamma[None, :]
    wk = w_qkv[1 * DIM:2 * DIM] * gamma[None, :]
    wv = w_qkv[2 * DIM:3 * DIM]

    def qk_rows(w):
        idx = (heads[:, None] * DH + _PERM_EO[None, :]).reshape(-1)
        return w[idx]

    v_rows = wv[(heads[:, None] * DH + np.arange(DH)[None, :]).reshape(-1)]
    wg_rows = w_gates[heads] * gamma[None, :]
    wqkv_t = np.concatenate([qk_rows(wq), qk_rows(wk), v_rows, wg_rows],
                            axis=0).T
    wout_t = w_out[:, heads[0] * DH:heads[0] * DH + HL * DH].T

    cos = rotary_cos[0, 0]  # (N, DH)
    sin = rotary_sin[0, 0]
    cosP = np.tile(np.concatenate([cos[:, 0::2], cos[:, 1::2]], axis=1), (1, 8))
    sinN = np.tile(np.concatenate([-sin[:, 0::2], sin[:, 1::2]], axis=1), (1, 8))

    bf = ml_dtypes.bfloat16
    return {
        "xT": np.ascontiguousarray(x[g].T).astype(bf),
        "xb": np.ascontiguousarray(x[g]).astype(bf),
        "wqkv": np.ascontiguousarray(wqkv_t).astype(bf),
        "wout": np.ascontiguousarray(wout_t).astype(bf),
        "cosP": np.ascontiguousarray(cosP).astype(bf),
        "sinN": np.ascontiguousarray(sinN).astype(bf),
        "bgn": np.tile(b_gates[heads][None, :], (P, 1)).astype(np.float32),
    }


def kernel(x, rotary_cos, rotary_sin, gamma, w_qkv, w_gates, b_gates, w_out):
    global _last_result
    args = [np.asarray(a, np.float32) for a in
            (x, rotary_cos, rotary_sin, gamma, w_qkv, w_gates, b_gates, w_out)]
    nc = _get_nc()
    in_maps = [_shard(c, *args) for c in range(CORES)]
    try:
        res = run_bass_kernel_spmd(
            nc, in_maps, core_ids=list(range(CORES)),
            trace=bool(os.environ.get("KTRACE")),
        )
    except ModuleNotFoundError:
        res = run_bass_kernel_spmd(nc, in_maps, core_ids=list(range(CORES)))
    _last_result = res
    full = np.zeros((B, N, DIM), np.float32)
    for c in range(CORES):
        g, r = c // 4, c % 4
        o = np.asarray(res.results[c]["out"]).astype(np.float32)
        for hf in range(2):
            full[g, hf * 1024 + r * 256: hf * 1024 + (r + 1) * 256, :] = o[hf]
    return full


# revision 13
# speedup vs baseline: 1.5700x; 1.0030x over previous
"""Distributed Trainium2 kernel for nn_Attention_61332132987140.

Gated multi-head attention block: RMSNorm -> QKV proj -> RoPE -> softmax
attention -> sigmoid head gating -> output projection.

Sharding: 8 cores = 2 batch groups x 4-head groups (tensor parallel on
heads). Each core computes attention for its batch's full sequence over
its 4 heads and the partial output projection; bf16 ReduceScatters over
each 4-core batch group sum the partials (quarters 0-2 in one collective,
quarter 3 in a small trailing one), leaving each core 512 tokens. The
host reassembles the full (2, 2048, 1024) output.

Cost-model-driven structure:
- activations arrive host-transposed (xT): no on-device xn transposes
- attention-V matmuls in [query, dim] orientation: full 128 output
  partitions per instruction (half the charge of [dim, query])
- softmax denominators as free-size-1 matmuls: ~130k PE cycles saved
- gates ride as 4 extra columns of the V projection; sigmoid via exp
- RMSNorm scale: ACT Square+accumulate for sum-sq, quake-rsqrt Newton on
  DVE (no Ln/Exp act-table switching), folded into RoPE via fused
  scalar_tensor_tensor (t = (qk*s)*cos in one DVE op)
- softmax exp split: ACT true exp + DVE Schraudolph-to-bf16-bits
  (bits = s*scale*log2e*128 + B, truncated to uint16, bitcast to bf16)
- PSUM accumulator banks (av/sums) hold several col-disjoint groups, so
  they are zeroed by K=1 zero matmuls and accumulated with start=False
  (start=True would mark the whole bank pending-zero and wipe siblings)
- per-quarter tail split in two: normalize+transpose emitted at the next
  quarter's start (frees psum bufs early), out-projection one jt-pair
  later (keeps PE streaming scores across the boundary)
"""
import os
import sys

sys.path.insert(0, "/opt/trn_rl_repo")

import numpy as np
import ml_dtypes

import concourse.bass as bass
import concourse.mybir as mybir
import concourse.tile as tile
from concourse import bacc
from concourse.bass_utils import run_bass_kernel_spmd

F32 = mybir.dt.float32
BF16 = mybir.dt.bfloat16
I32 = mybir.dt.int32
U16 = mybir.dt.uint16
AF = mybir.ActivationFunctionType
ALU = mybir.AluOpType

B, N, DIM = 2, 2048, 1024
HEADS, DH = 16, 64
HL = 4  # local heads per core
P = 128
TT = N // P  # 16 token tiles
KD = DIM // P  # 8 contraction tiles
NQ = 4  # quarters (512-query chunks)
CORES = 8
REPLICA_GROUPS = [[0, 1, 2, 3], [4, 5, 6, 7]]

_nc_cache = None
_last_result = None


def _build():
    nc = bacc.Bacc("TRN2", target_bir_lowering=False, debug=False, num_devices=CORES)

    xT_ext = nc.declare_dram_parameter("xT", [DIM, N], BF16, isOutput=False)
    xb_ext = nc.declare_dram_parameter("xb", [N, DIM], BF16, isOutput=False)
    wqkv_ext = nc.declare_dram_parameter("wqkv", [DIM, 772], BF16, isOutput=False)
    wout_ext = nc.declare_dram_parameter("wout", [2 * P, DIM], BF16, isOutput=False)
    cosP_ext = nc.declare_dram_parameter("cosP", [N, 512], BF16, isOutput=False)
    sinN_ext = nc.declare_dram_parameter("sinN", [N, 512], BF16, isOutput=False)
    bgn_ext = nc.declare_dram_parameter("bgn", [P, HL], F32, isOutput=False)
    out_ext = nc.declare_dram_parameter("out", [4 * P, DIM], BF16, isOutput=True)

    with tile.TileContext(nc) as tc:
        with (
            tc.tile_pool(name="wpool", bufs=1) as wpool,
            tc.tile_pool(name="persist", bufs=1) as persist,
            tc.tile_pool(name="xbp", bufs=4) as xbp,
            tc.tile_pool(name="sqp", bufs=2) as sqp,
            tc.tile_pool(name="small", bufs=4) as small,
            tc.tile_pool(name="tup", bufs=6) as tup,
            tc.tile_pool(name="qksp", bufs=4) as qksp,
            tc.tile_pool(name="ptp", bufs=14) as ptp,
            tc.tile_pool(name="op", bufs=2) as op_pool,
            tc.tile_pool(name="otp", bufs=2) as otp,
            tc.tile_pool(name="ysq", bufs=2) as ysq,
            tc.tile_pool(name="ps_s", bufs=3, space="PSUM") as ps_s,
            tc.tile_pool(name="ps_qy", bufs=2, space="PSUM") as ps_qy,
            tc.tile_pool(name="ps_va", bufs=2, space="PSUM") as ps_va,
            tc.tile_pool(name="dram", bufs=1, space="DRAM") as dramp,
        ):
            # ---- constants / weights ----
            wqkv_sb = wpool.tile([P, KD, 772], BF16)
            nc.scalar.dma_start(wqkv_sb[:], wqkv_ext.rearrange("(k p) f -> p k f", p=P))
            wout_sb = wpool.tile([P, 2, DIM], BF16)
            nc.scalar.dma_start(wout_sb[:], wout_ext.rearrange("(k p) f -> p k f", p=P))
            bgn_sb = wpool.tile([P, HL], F32)
            nc.scalar.dma_start(bgn_sb[:], bgn_ext[:])
            cosP_sb = wpool.tile([P, TT, 512], BF16)
            nc.scalar.dma_start(cosP_sb[:], cosP_ext.rearrange("(t p) f -> p t f", p=P))
            sinN_sb = wpool.tile([P, TT, 512], BF16)
            nc.scalar.dma_start(sinN_sb[:], sinN_ext.rearrange("(t p) f -> p t f", p=P))
            zb = wpool.tile([P, 1], F32)
            nc.gpsimd.memset(zb[:], 0.0)
            ones_mm = wpool.tile([P, 1], BF16)
            nc.gpsimd.memset(ones_mm[:], 1.0)
            ones1 = wpool.tile([1, P], BF16)
            nc.gpsimd.memset(ones1[:], 1.0)
            zrow = wpool.tile([1, 512], BF16)
            nc.gpsimd.memset(zrow[:], 0.0)

            # ---- persistent activations ----
            xT_sb = persist.tile([P, KD, N], BF16)
            for c in range(4):
                nc.gpsimd.dma_start(
                    xT_sb[:, :, c * 512:(c + 1) * 512],
                    xT_ext.rearrange("(k p) t -> p k t", p=P)[:, :, c * 512:(c + 1) * 512],
                )
            # qkT blocks: 0=q(h0,h1) 1=q(h2,h3) 2=k(h0,h1) 3=k(h2,h3); rows=dh
            qkT_sb = persist.tile([P, 4, N], BF16)
            v_sb = persist.tile([P, TT, 256], BF16)
            gates_sb = persist.tile([P, TT, HL], F32)
            ss_all = persist.tile([P, TT], F32)
            s_all = persist.tile([P, TT], F32)

            def emit_norm_group(g):
                """sum-sq (ACT Square+accum) + quake rsqrt for tiles 4g..4g+3.

                s = 32*ss^-0.5 with the /1024 folded into the quake seed's
                pre-scale; two Newton iterations on DVE. No act-table loads.
                """
                for tt in range(4 * g, 4 * g + 4):
                    xb_t = xbp.tile([P, DIM], BF16, name="xb_t")
                    nc.sync.dma_start(xb_t[:], xb_ext[tt * P:(tt + 1) * P, :])
                    scr = sqp.tile([P, DIM], BF16, name="scr")
                    nc.scalar.activation(scr[:], xb_t[:], AF.Square,
                                         accum_out=ss_all[:, tt:tt + 1])
                gs = slice(4 * g, 4 * g + 4)
                xp = small.tile([P, 4], F32, name="xp")
                nc.vector.tensor_scalar(out=xp[:], in0=ss_all[:, gs],
                                        scalar1=1.0 / 1024.0, scalar2=None,
                                        op0=ALU.mult)
                jt_ = small.tile([P, 4], I32, name="jt_")
                nc.vector.tensor_scalar(out=jt_[:], in0=xp[:].bitcast(I32),
                                        scalar1=1, scalar2=None,
                                        op0=ALU.logical_shift_right)
                y0b = small.tile([P, 4], I32, name="y0b")
                nc.vector.tensor_scalar(out=y0b[:], in0=jt_[:], scalar1=-1,
                                        scalar2=0x5f3759df, op0=ALU.mult,
                                        op1=ALU.add)
                cur = y0b[:].bitcast(F32)
                for it in range(2):
                    h_ = small.tile([P, 4], F32, name="h_")
                    nc.vector.tensor_tensor(out=h_[:], in0=cur, in1=cur,
                                            op=ALU.mult)
                    hx = small.tile([P, 4], F32, name="hx")
                    nc.vector.tensor_tensor(out=hx[:], in0=h_[:], in1=xp[:],
                                            op=ALU.mult)
                    w_ = small.tile([P, 4], F32, name="w_")
                    nc.vector.tensor_scalar(out=w_[:], in0=hx[:], scalar1=-0.5,
                                            scalar2=1.5, op0=ALU.mult,
                                            op1=ALU.add)
                    dst = s_all[:, gs] if it == 1 else \
                        small.tile([P, 4], F32, name="nx")[:]
                    nc.vector.tensor_tensor(out=dst, in0=cur, in1=w_[:],
                                            op=ALU.mult)
                    cur = dst

            # ---- phase A per token tile: QKV + RoPE + transposes + gates ----
            for tt in range(TT):
                if tt % 4 == 0:
                    emit_norm_group(tt // 4)
                s_ap = s_all[:, tt:tt + 1]
                qk_ps = ps_qy.tile([P, 512], F32, name="qk_ps", tag="qy")
                vg_ps = ps_va.tile([P, 260], F32, name="vg_ps", tag="va")
                for kd in range(KD):
                    lhsT = xT_sb[:, kd, tt * P:(tt + 1) * P]
                    nc.tensor.matmul(qk_ps[:], lhsT, wqkv_sb[:, kd, 0:512],
                                     start=(kd == 0), stop=(kd == KD - 1))
                    nc.tensor.matmul(vg_ps[:], lhsT, wqkv_sb[:, kd, 512:772],
                                     start=(kd == 0), stop=(kd == KD - 1))

                # rope with norm scale fused: qk' = (qk*s)*cos + (swap*s)*sin
                t_sb = tup.tile([P, 512], BF16, name="t_sb")
                nc.vector.scalar_tensor_tensor(
                    out=t_sb[:], in0=qk_ps[:], scalar=s_ap,
                    in1=cosP_sb[:, tt, :], op0=ALU.mult, op1=ALU.mult)
                u_sb = tup.tile([P, 512], BF16, name="u_sb")
                qkv8 = qk_ps[:].rearrange("p (b c) -> p b c", b=8)
                u8 = u_sb[:].rearrange("p (b c) -> p b c", b=8)
                sin8 = sinN_sb[:, tt, :].rearrange("p (b c) -> p b c", b=8)
                nc.vector.scalar_tensor_tensor(
                    out=u8[:, :, 0:32], in0=qkv8[:, :, 32:64], scalar=s_ap,
                    in1=sin8[:, :, 0:32], op0=ALU.mult, op1=ALU.mult)
                nc.vector.scalar_tensor_tensor(
                    out=u8[:, :, 32:64], in0=qkv8[:, :, 0:32], scalar=s_ap,
                    in1=sin8[:, :, 32:64], op0=ALU.mult, op1=ALU.mult)
                qk_sb = qksp.tile([P, 512], BF16, name="qk_sb")
                nc.gpsimd.tensor_tensor(out=qk_sb[:], in0=t_sb[:], in1=u_sb[:],
                                        op=ALU.add)
                nc.sync.dma_start_transpose(qkT_sb[:, :, tt * P:(tt + 1) * P], qk_sb[:])

                # v with norm scale
                nc.vector.tensor_scalar(out=v_sb[:, tt, :], in0=vg_ps[:, 0:256],
                                        scalar1=s_ap, scalar2=None, op0=ALU.mult)

                # gates: sigmoid(s*z + b) via exp
                zt = small.tile([P, HL], BF16, name="zt")
                nc.vector.scalar_tensor_tensor(out=zt[:], in0=vg_ps[:, 256:260],
                                               scalar=s_ap, in1=bgn_sb[:],
                                               op0=ALU.mult, op1=ALU.add)
                ge = small.tile([P, HL], F32, name="ge")
                nc.scalar.activation(ge[:], zt[:], AF.Exp, scale=-1.0, bias=zb[:])
                gp = small.tile([P, HL], F32, name="gp")
                nc.vector.tensor_scalar_add(gp[:], ge[:], 1.0)
                nc.vector.reciprocal(gates_sb[:, tt, :], gp[:])

            # ---- phase B: attention + out proj + RS ----
            ydram_a = dramp.tile([3 * 512, DIM], BF16, name="ydram_a")
            ydram_b = dramp.tile([512, DIM], BF16, name="ydram_b")
            rsout_a = dramp.tile([3 * P, DIM], BF16, name="rsout_a")
            rsout_b = dramp.tile([P, DIM], BF16, name="rsout_b")

            # Schraudolph exp-to-bf16 bits: bits = s*0.125*log2e*128 + B
            SCH_A = float(0.125 * np.log2(np.e) * 128.0)
            SCH_B = float(16256.5 - 5.5)

            def emit_scores(ci, m):
                """scores+exp for jt pair (2m, 2m+1), all 4 heads."""
                pts = []
                for h in range(4):
                    pt = ptp.tile([P, 2, 512], BF16, name="pt")
                    for par in range(2):
                        jt = 2 * m + par
                        hh = 64 * (h % 2)
                        s_t = ps_s.tile([P, 512], F32, name="s_t", tag="s")
                        nc.tensor.matmul(
                            s_t[:],
                            qkT_sb[hh:hh + 64, 2 + h // 2, jt * P:(jt + 1) * P],
                            qkT_sb[hh:hh + 64, h // 2, ci * 512:(ci + 1) * 512],
                            start=True, stop=True,
                        )
                        if jt % 16 in (1, 3, 5, 7, 9, 11, 13):
                            nc.vector.tensor_scalar(
                                out=pt[:, par, :].bitcast(U16),
                                in0=s_t[:], scalar1=SCH_A, scalar2=SCH_B,
                                op0=ALU.mult, op1=ALU.add)
                        else:
                            nc.scalar.activation(pt[:, par, :], s_t[:], AF.Exp,
                                                 scale=0.125, bias=zb[:])
                    pts.append(pt)
                return pts

            def emit_av(m, pts, av_t, sums_t):
                for h in range(4):
                    for qt in range(4):
                        for par in range(2):
                            jt = 2 * m + par
                            nc.tensor.matmul(
                                av_t[qt][:, h * DH:(h + 1) * DH],
                                pts[h][:, par, qt * P:(qt + 1) * P],
                                v_sb[:, jt, h * DH:(h + 1) * DH],
                                start=False, stop=(m == TT // 2 - 1 and par == 1),
                                skip_group_check=True,
                            )
                            nc.tensor.matmul(
                                sums_t[:, (h * 4 + qt):(h * 4 + qt) + 1],
                                pts[h][:, par, qt * P:(qt + 1) * P],
                                ones_mm[:, 0:1],
                                start=False, stop=(m == TT // 2 - 1 and par == 1),
                                skip_group_check=True,
                            )

            def emit_tail_pre(ci, av_pair, sums_t):
                """normalize + gate + transpose; frees av/sums psum bufs."""
                av_t = [av_pair[qt // 2][:, qt % 2, :] for qt in range(4)]
                oT = otp.tile([P, 2, 512], BF16, name="oT")
                for qt in range(4):
                    rec = small.tile([P, HL], F32, name="rec")
                    nc.vector.reciprocal(
                        rec[:],
                        sums_t[:, 0:16].rearrange("p (h q) -> p q h", q=4)[:, qt, :])
                    scl = small.tile([P, HL], F32, name="scl")
                    nc.vector.tensor_tensor(out=scl[:], in0=rec[:],
                                            in1=gates_sb[:, ci * 4 + qt, :],
                                            op=ALU.mult)
                    o_sb = op_pool.tile([P, 256], BF16, name="o_sb")
                    for h in range(4):
                        nc.vector.tensor_scalar(
                            out=o_sb[:, h * DH:(h + 1) * DH],
                            in0=av_t[qt][:, h * DH:(h + 1) * DH],
                            scalar1=scl[:, h:h + 1], scalar2=None, op0=ALU.mult)
                    nc.sync.dma_start_transpose(oT[:, :, qt * P:(qt + 1) * P],
                                                o_sb[:])
                return oT

            def emit_tail_y(ci, oT, ydst):
                """out-projection + psum copy + ydram store for quarter ci."""
                y_sbq = ysq.tile([P, 4, DIM], BF16, name="y_sbq")
                for qt in range(4):
                    for oh in range(2):
                        y_ps = ps_qy.tile([P, 512], F32, name="y_ps", tag="qy")
                        for kh in range(2):
                            nc.tensor.matmul(
                                y_ps[:],
                                oT[:, kh, qt * P:(qt + 1) * P],
                                wout_sb[:, kh, oh * 512:(oh + 1) * 512],
                                start=(kh == 0), stop=(kh == 1),
                            )
                        if oh == 0:
                            nc.scalar.activation(
                                y_sbq[:, qt, oh * 512:(oh + 1) * 512], y_ps[:],
                                AF.Copy, bias=0.0)
                        else:
                            nc.vector.tensor_copy(
                                y_sbq[:, qt, oh * 512:(oh + 1) * 512], y_ps[:])
                nc.gpsimd.dma_start(ydst, y_sbq[:])

            def ydst_for(ci):
                if ci < 3:
                    return ydram_a[ci * 512:(ci + 1) * 512, :] \
                        .rearrange("(q p) o -> p q o", p=P)
                return ydram_b[:].rearrange("(q p) o -> p q o", p=P)

            pending = None  # (ci, av_pair, sums_t) awaiting tail emission
            oT_prev = None
            for ci in range(NQ):
                if pending is not None:
                    oT_prev = emit_tail_pre(*pending)
                av_pair = [ps_va.tile([P, 2, 256], F32, name=f"av{i}", tag="va")
                           for i in range(2)]
                av_t = [av_pair[qt // 2][:, qt % 2, :] for qt in range(4)]
                sums_t = ps_s.tile([P, 16], F32, name="sums_t", tag="sums", bufs=1)
                # zero accumulator banks via K=1 zero matmuls (see docstring)
                for i in range(2):
                    nc.tensor.matmul(av_pair[i][:], ones1[:], zrow[:],
                                     start=True, stop=True)
                nc.tensor.matmul(sums_t[:], ones1[:], zrow[:, 0:16],
                                 start=True, stop=True)
                for m in range(TT // 2):
                    pts = emit_scores(ci, m)
                    emit_av(m, pts, av_t, sums_t)
                    if m == 1 and pending is not None:
                        emit_tail_y(pending[0], oT_prev, ydst_for(pending[0]))
                        if pending[0] == 2:
                            nc.gpsimd.collective_compute(
                                "ReduceScatter", ALU.add,
                                replica_groups=REPLICA_GROUPS,
                                ins=[ydram_a[:].opt()],
                                outs=[rsout_a[:].opt()],
                            )
                            nc.gpsimd.dma_start(out_ext[0:3 * P, :], rsout_a[:])
                        pending = None
                pending = (ci, av_pair, sums_t)

            oT_last = emit_tail_pre(*pending)
            emit_tail_y(pending[0], oT_last, ydst_for(pending[0]))
            nc.gpsimd.collective_compute(
                "ReduceScatter", ALU.add,
                replica_groups=REPLICA_GROUPS,
                ins=[ydram_b[:].opt()],
                outs=[rsout_b[:].opt()],
            )
            nc.gpsimd.dma_start(out_ext[3 * P:4 * P, :], rsout_b[:])

    nc.compile()
    return nc


def _get_nc():
    global _nc_cache
    if _nc_cache is None:
        _nc_cache = _build()
    return _nc_cache


_PERM_EO = np.concatenate([np.arange(0, DH, 2), np.arange(1, DH, 2)])


def _shard(core, x, rotary_cos, rotary_sin, gamma, w_qkv, w_gates, b_gates, w_out):
    g, r = core // 4, core % 4
    heads = np.arange(4 * r, 4 * r + 4)
    wq = w_qkv[0 * DIM:1 * DIM] * gamma[None, :]
    wk = w_qkv[1 * DIM:2 * DIM] * gamma[None, :]
    wv = w_qkv[2 * DIM:3 * DIM]

    def qk_rows(w):
        idx = (heads[:, None] * DH + _PERM_EO[None, :]).reshape(-1)
        return w[idx]

    v_rows = wv[(heads[:, None] * DH + np.arange(DH)[None, :]).reshape(-1)]
    wg_rows = w_gates[heads] * gamma[None, :]
    wqkv_t = np.concatenate([qk_rows(wq), qk_rows(wk), v_rows, wg_rows],
                            axis=0).T
    wout_t = w_out[:, heads[0] * DH:heads[0] * DH + HL * DH].T

    cos = rotary_cos[0, 0]  # (N, DH)
    sin = rotary_sin[0, 0]
    cosP = np.tile(np.concatenate([cos[:, 0::2], cos[:, 1::2]], axis=1), (1, 8))
    sinN = np.tile(np.concatenate([-sin[:, 0::2], sin[:, 1::2]], axis=1), (1, 8))

    bf = ml_dtypes.bfloat16
    return {
        "xT": np.ascontiguousarray(x[g].T).astype(bf),
        "xb": np.ascontiguousarray(x[g]).astype(bf),
        "wqkv": np.ascontiguousarray(wqkv_t).astype(bf),
        "wout": np.ascontiguousarray(wout_t).astype(bf),
        "cosP": np.ascontiguousarray(cosP).astype(bf),
        "sinN": np.ascontiguousarray(sinN).astype(bf),
        "bgn": np.tile(b_gates[heads][None, :], (P, 1)).astype(np.float32),
    }


def kernel(x, rotary_cos, rotary_sin, gamma, w_qkv, w_gates, b_gates, w_out):
    global _last_result
    args = [np.asarray(a, np.float32) for a in
            (x, rotary_cos, rotary_sin, gamma, w_qkv, w_gates, b_gates, w_out)]
    nc = _get_nc()
    in_maps = [_shard(c, *args) for c in range(CORES)]
    try:
        res = run_bass_kernel_spmd(
            nc, in_maps, core_ids=list(range(CORES)),
            trace=bool(os.environ.get("KTRACE")),
        )
    except ModuleNotFoundError:
        res = run_bass_kernel_spmd(nc, in_maps, core_ids=list(range(CORES)))
    _last_result = res
    full = np.zeros((B, N, DIM), np.float32)
    for c in range(CORES):
        g, r = c // 4, c % 4
        o = np.asarray(res.results[c]["out"]).astype(np.float32)
        full[g, r * 384:(r + 1) * 384, :] = o[0:384]
        full[g, 1536 + r * P:1536 + (r + 1) * P, :] = o[384:512]
    return full


# revision 16
# speedup vs baseline: 1.7745x; 1.1303x over previous
"""Distributed Trainium2 kernel for nn_Attention_61332132987140.

Gated multi-head attention block: RMSNorm -> QKV proj -> RoPE -> softmax
attention -> sigmoid head gating -> output projection.

Sharding: 8 cores = 2 batch groups x 4-head groups (tensor parallel on
heads). Each core computes attention for its batch's full sequence over
its 4 heads and the partial output projection; bf16 ReduceScatters over
each 4-core batch group sum the partials (quarters 0-2 in one collective,
quarter 3 in a small trailing one), leaving each core 512 tokens. The
host reassembles the full (2, 2048, 1024) output.

Cost-model-driven structure:
- activations arrive host-transposed (xT): no on-device xn transposes
- attention-V matmuls in [query, dim] orientation: full 128 output
  partitions per instruction (half the charge of [dim, query])
- softmax denominators as free-size-1 matmuls: ~130k PE cycles saved
- gates ride as 4 extra columns of the V projection; sigmoid via exp
- RMSNorm scale: ACT Square+accumulate for sum-sq, quake-rsqrt Newton on
  DVE (no Ln/Exp act-table switching), folded into RoPE via fused
  scalar_tensor_tensor (t = (qk*s)*cos in one DVE op)
- softmax exp split: ACT true exp + DVE Schraudolph-to-bf16-bits
  (bits = s*scale*log2e*128 + B, truncated to uint16, bitcast to bf16)
- PSUM accumulator banks (av/sums) hold several col-disjoint groups, so
  they are zeroed by K=1 zero matmuls and accumulated with start=False
  (start=True would mark the whole bank pending-zero and wipe siblings)
- per-quarter tail split in two: normalize+transpose emitted at the next
  quarter's start (frees psum bufs early), out-projection one jt-pair
  later (keeps PE streaming scores across the boundary)
"""
import os
import sys

sys.path.insert(0, "/opt/trn_rl_repo")

import numpy as np
import ml_dtypes

import concourse.bass as bass
import concourse.mybir as mybir
import concourse.tile as tile
from concourse import bacc
from concourse.bass_utils import run_bass_kernel_spmd

F32 = mybir.dt.float32
BF16 = mybir.dt.bfloat16
I32 = mybir.dt.int32
U16 = mybir.dt.uint16
AF = mybir.ActivationFunctionType
ALU = mybir.AluOpType

B, N, DIM = 2, 2048, 1024
HEADS, DH = 16, 64
HL = 4  # local heads per core
P = 128
TT = N // P  # 16 token tiles
KD = DIM // P  # 8 contraction tiles
NQ = 4  # quarters (512-query chunks)
CORES = 8
REPLICA_GROUPS = [[0, 1, 2, 3], [4, 5, 6, 7]]

_nc_cache = None
_last_result = None


def _build():
    nc = bacc.Bacc("TRN2", target_bir_lowering=False, debug=False, num_devices=CORES)

    xT_ext = nc.declare_dram_parameter("xT", [DIM, N], BF16, isOutput=False)
    xb_ext = nc.declare_dram_parameter("xb", [N, DIM], BF16, isOutput=False)
    wqkv_ext = nc.declare_dram_parameter("wqkv", [DIM, 772], BF16, isOutput=False)
    wout_ext = nc.declare_dram_parameter("wout", [2 * P, DIM], BF16, isOutput=False)
    cosP_ext = nc.declare_dram_parameter("cosP", [N, 512], BF16, isOutput=False)
    sinN_ext = nc.declare_dram_parameter("sinN", [N, 512], BF16, isOutput=False)
    bgn_ext = nc.declare_dram_parameter("bgn", [P, HL], F32, isOutput=False)
    out_ext = nc.declare_dram_parameter("out", [4 * P, DIM], BF16, isOutput=True)

    OUTCOPY_ENG = {"sp": nc.sync, "act": nc.scalar}.get(
        os.environ.get("KCOPYQ", ""), nc.gpsimd)
    with tile.TileContext(nc) as tc:
        with (
            tc.tile_pool(name="wpool", bufs=1) as wpool,
            tc.tile_pool(name="persist", bufs=1) as persist,
            tc.tile_pool(name="xbp", bufs=4) as xbp,
            tc.tile_pool(name="sqp", bufs=2) as sqp,
            tc.tile_pool(name="small", bufs=4) as small,
            tc.tile_pool(name="tup", bufs=6) as tup,
            tc.tile_pool(name="qksp", bufs=4) as qksp,
            tc.tile_pool(name="ptp", bufs=14) as ptp,
            tc.tile_pool(name="op", bufs=2) as op_pool,
            tc.tile_pool(name="otp", bufs=2) as otp,
            tc.tile_pool(name="ysq", bufs=2) as ysq,
            tc.tile_pool(name="ps_s", bufs=3, space="PSUM") as ps_s,
            tc.tile_pool(name="ps_qy", bufs=2, space="PSUM") as ps_qy,
            tc.tile_pool(name="ps_va", bufs=2, space="PSUM") as ps_va,
            tc.tile_pool(name="dram", bufs=1, space="DRAM") as dramp,
        ):
            # ---- constants / weights (loads emitted later, ordered for
            # startup: xb/xT first, weights next, tables per group) ----
            wqkv_sb = wpool.tile([P, KD, 772], BF16)
            wout_sb = wpool.tile([P, 2, DIM], BF16)
            bgn_sb = wpool.tile([P, HL], F32)
            cosP_sb = wpool.tile([P, TT, 512], BF16)
            sinN_sb = wpool.tile([P, TT, 512], BF16)
            zb = wpool.tile([P, 1], F32)
            nc.gpsimd.memset(zb[:], 0.0)
            ones_mm = wpool.tile([P, 1], BF16)
            nc.gpsimd.memset(ones_mm[:], 1.0)
            ones1 = wpool.tile([1, P], BF16)
            nc.gpsimd.memset(ones1[:], 1.0)
            zrow = wpool.tile([1, 512], BF16)
            nc.gpsimd.memset(zrow[:], 0.0)

            # ---- persistent activations ----
            xT_sb = persist.tile([P, KD, N], BF16)
            # qkT blocks: 0=q(h0,h1) 1=q(h2,h3) 2=k(h0,h1) 3=k(h2,h3); rows=dh
            qkT_sb = persist.tile([P, 4, N], BF16)
            v_sb = persist.tile([P, TT, 256], BF16)
            gates_sb = persist.tile([P, TT, HL], F32)
            ss_all = persist.tile([P, TT], F32)
            s_all = persist.tile([P, TT], F32)

            def emit_norm_group(g):
                """sum-sq (ACT Square+accum) + quake rsqrt for tiles 4g..4g+3.

                s = 32*ss^-0.5 with the /1024 folded into the quake seed's
                pre-scale; two Newton iterations on DVE. No act-table loads.
                """
                for tt in range(4 * g, 4 * g + 4):
                    xb_t = xbp.tile([P, DIM], BF16, name="xb_t")
                    nc.sync.dma_start(xb_t[:], xb_ext[tt * P:(tt + 1) * P, :])
                    scr = sqp.tile([P, DIM], BF16, name="scr")
                    nc.scalar.activation(scr[:], xb_t[:], AF.Square,
                                         accum_out=ss_all[:, tt:tt + 1])
                gs = slice(4 * g, 4 * g + 4)
                xp = small.tile([P, 4], F32, name="xp")
                nc.vector.tensor_scalar(out=xp[:], in0=ss_all[:, gs],
                                        scalar1=1.0 / 1024.0, scalar2=None,
                                        op0=ALU.mult)
                jt_ = small.tile([P, 4], I32, name="jt_")
                nc.vector.tensor_scalar(out=jt_[:], in0=xp[:].bitcast(I32),
                                        scalar1=1, scalar2=None,
                                        op0=ALU.logical_shift_right)
                y0b = small.tile([P, 4], I32, name="y0b")
                nc.vector.tensor_scalar(out=y0b[:], in0=jt_[:], scalar1=-1,
                                        scalar2=0x5f3759df, op0=ALU.mult,
                                        op1=ALU.add)
                cur = y0b[:].bitcast(F32)
                for it in range(2):
                    h_ = small.tile([P, 4], F32, name="h_")
                    nc.vector.tensor_tensor(out=h_[:], in0=cur, in1=cur,
                                            op=ALU.mult)
                    hx = small.tile([P, 4], F32, name="hx")
                    nc.vector.tensor_tensor(out=hx[:], in0=h_[:], in1=xp[:],
                                            op=ALU.mult)
                    w_ = small.tile([P, 4], F32, name="w_")
                    nc.vector.tensor_scalar(out=w_[:], in0=hx[:], scalar1=-0.5,
                                            scalar2=1.5, op0=ALU.mult,
                                            op1=ALU.add)
                    dst = s_all[:, gs] if it == 1 else \
                        small.tile([P, 4], F32, name="nx")[:]
                    nc.vector.tensor_tensor(out=dst, in0=cur, in1=w_[:],
                                            op=ALU.mult)
                    cur = dst

            def emit_tables_group(g):
                gs = slice(4 * g, 4 * g + 4)
                nc.scalar.dma_start(
                    cosP_sb[:, gs, :],
                    cosP_ext.rearrange("(t p) f -> p t f", p=P)[:, gs, :])
                nc.scalar.dma_start(
                    sinN_sb[:, gs, :],
                    sinN_ext.rearrange("(t p) f -> p t f", p=P)[:, gs, :])

            # startup order: norm group 0 (xb loads) -> xT chunk0 -> wqkv ->
            # tables group0 -> remaining xT/xb/tables interleaved per group
            emit_norm_group(0)
            xT_re = xT_ext.rearrange("(k p) t -> p k t", p=P)
            nc.gpsimd.dma_start(xT_sb[:, :, 0:512], xT_re[:, :, 0:512])
            nc.scalar.dma_start(wqkv_sb[:], wqkv_ext.rearrange("(k p) f -> p k f", p=P))
            emit_tables_group(0)
            nc.scalar.dma_start(bgn_sb[:], bgn_ext[:])
            for c in range(1, 4):
                nc.gpsimd.dma_start(
                    xT_sb[:, :, c * 512:(c + 1) * 512], xT_re[:, :, c * 512:(c + 1) * 512])

            # ---- phase A per token tile: QKV + RoPE + transposes + gates ----
            for tt in range(TT):
                if tt % 4 == 0 and tt > 0:
                    emit_norm_group(tt // 4)
                    emit_tables_group(tt // 4)
                s_ap = s_all[:, tt:tt + 1]
                qk_ps = ps_qy.tile([P, 512], F32, name="qk_ps", tag="qy")
                vg_ps = ps_va.tile([P, 260], F32, name="vg_ps", tag="va")
                for kd in range(KD):
                    lhsT = xT_sb[:, kd, tt * P:(tt + 1) * P]
                    nc.tensor.matmul(qk_ps[:], lhsT, wqkv_sb[:, kd, 0:512],
                                     start=(kd == 0), stop=(kd == KD - 1))
                    nc.tensor.matmul(vg_ps[:], lhsT, wqkv_sb[:, kd, 512:772],
                                     start=(kd == 0), stop=(kd == KD - 1))

                # rope with norm scale fused: qk' = (qk*s)*cos + (swap*s)*sin
                t_sb = tup.tile([P, 512], BF16, name="t_sb")
                nc.vector.scalar_tensor_tensor(
                    out=t_sb[:], in0=qk_ps[:], scalar=s_ap,
                    in1=cosP_sb[:, tt, :], op0=ALU.mult, op1=ALU.mult)
                u_sb = tup.tile([P, 512], BF16, name="u_sb")
                qkv8 = qk_ps[:].rearrange("p (b c) -> p b c", b=8)
                u8 = u_sb[:].rearrange("p (b c) -> p b c", b=8)
                sin8 = sinN_sb[:, tt, :].rearrange("p (b c) -> p b c", b=8)
                nc.vector.scalar_tensor_tensor(
                    out=u8[:, :, 0:32], in0=qkv8[:, :, 32:64], scalar=s_ap,
                    in1=sin8[:, :, 0:32], op0=ALU.mult, op1=ALU.mult)
                nc.vector.scalar_tensor_tensor(
                    out=u8[:, :, 32:64], in0=qkv8[:, :, 0:32], scalar=s_ap,
                    in1=sin8[:, :, 32:64], op0=ALU.mult, op1=ALU.mult)
                qk_sb = qksp.tile([P, 512], BF16, name="qk_sb")
                nc.gpsimd.tensor_tensor(out=qk_sb[:], in0=t_sb[:], in1=u_sb[:],
                                        op=ALU.add)
                nc.sync.dma_start_transpose(qkT_sb[:, :, tt * P:(tt + 1) * P], qk_sb[:])

                # v with norm scale
                nc.vector.tensor_scalar(out=v_sb[:, tt, :], in0=vg_ps[:, 0:256],
                                        scalar1=s_ap, scalar2=None, op0=ALU.mult)

                # gates: sigmoid(s*z + b) via exp
                zt = small.tile([P, HL], BF16, name="zt")
                nc.vector.scalar_tensor_tensor(out=zt[:], in0=vg_ps[:, 256:260],
                                               scalar=s_ap, in1=bgn_sb[:],
                                               op0=ALU.mult, op1=ALU.add)
                ge = small.tile([P, HL], F32, name="ge")
                nc.scalar.activation(ge[:], zt[:], AF.Exp, scale=-1.0, bias=zb[:])
                gp = small.tile([P, HL], F32, name="gp")
                nc.vector.tensor_scalar_add(gp[:], ge[:], 1.0)
                nc.vector.reciprocal(gates_sb[:, tt, :], gp[:])

            nc.scalar.dma_start(wout_sb[:], wout_ext.rearrange("(k p) f -> p k f", p=P))

            # ---- phase B: attention + out proj + RS ----
            ydram_a = dramp.tile([3 * 512, DIM], BF16, name="ydram_a", tag="yda")
            ydram_b = dramp.tile([512, DIM], BF16, name="ydram_b", tag="ydb")
            rsout_a = dramp.tile([3 * P, DIM], BF16, name="rsout_a", tag="rsa")
            rsout_b = dramp.tile([P, DIM], BF16, name="rsout_b", tag="rsb")

            # Schraudolph exp-to-bf16 bits: bits = s*0.125*log2e*128 + B
            SCH_A = float(0.125 * np.log2(np.e) * 128.0)
            SCH_B = float(16256.5 - 5.5)

            def emit_scores(ci, m):
                """scores+exp for jt pair (2m, 2m+1), all 4 heads."""
                pts = []
                for h in range(4):
                    pt = ptp.tile([P, 2, 512], BF16, name="pt")
                    for par in range(2):
                        jt = 2 * m + par
                        hh = 64 * (h % 2)
                        s_t = ps_s.tile([P, 512], F32, name="s_t", tag="s")
                        nc.tensor.matmul(
                            s_t[:],
                            qkT_sb[hh:hh + 64, 2 + h // 2, jt * P:(jt + 1) * P],
                            qkT_sb[hh:hh + 64, h // 2, ci * 512:(ci + 1) * 512],
                            start=True, stop=True,
                        )
                        if jt % 16 in (1, 3, 5, 7, 9, 11, 13):
                            nc.vector.tensor_scalar(
                                out=pt[:, par, :].bitcast(U16),
                                in0=s_t[:], scalar1=SCH_A, scalar2=SCH_B,
                                op0=ALU.mult, op1=ALU.add)
                        else:
                            nc.scalar.activation(pt[:, par, :], s_t[:], AF.Exp,
                                                 scale=0.125, bias=zb[:])
                    pts.append(pt)
                return pts

            def emit_av(m, pts, av_t, sums_t):
                for h in range(4):
                    for qt in range(4):
                        for par in range(2):
                            jt = 2 * m + par
                            nc.tensor.matmul(
                                av_t[qt][:, h * DH:(h + 1) * DH],
                                pts[h][:, par, qt * P:(qt + 1) * P],
                                v_sb[:, jt, h * DH:(h + 1) * DH],
                                start=False, stop=(m == TT // 2 - 1 and par == 1),
                                skip_group_check=True,
                            )
                            nc.tensor.matmul(
                                sums_t[:, (h * 4 + qt):(h * 4 + qt) + 1],
                                pts[h][:, par, qt * P:(qt + 1) * P],
                                ones_mm[:, 0:1],
                                start=False, stop=(m == TT // 2 - 1 and par == 1),
                                skip_group_check=True,
                            )

            def emit_tail_pre(ci, av_pair, sums_t):
                """normalize + gate + transpose; frees av/sums psum bufs."""
                av_t = [av_pair[qt // 2][:, qt % 2, :] for qt in range(4)]
                oT = otp.tile([P, 2, 512], BF16, name="oT")
                for qt in range(4):
                    rec = small.tile([P, HL], F32, name="rec")
                    nc.vector.reciprocal(
                        rec[:],
                        sums_t[:, 0:16].rearrange("p (h q) -> p q h", q=4)[:, qt, :])
                    scl = small.tile([P, HL], F32, name="scl")
                    nc.vector.tensor_tensor(out=scl[:], in0=rec[:],
                                            in1=gates_sb[:, ci * 4 + qt, :],
                                            op=ALU.mult)
                    o_sb = op_pool.tile([P, 256], BF16, name="o_sb")
                    for h in range(4):
                        nc.vector.tensor_scalar(
                            out=o_sb[:, h * DH:(h + 1) * DH],
                            in0=av_t[qt][:, h * DH:(h + 1) * DH],
                            scalar1=scl[:, h:h + 1], scalar2=None, op0=ALU.mult)
                    nc.sync.dma_start_transpose(oT[:, :, qt * P:(qt + 1) * P],
                                                o_sb[:])
                return oT

            def emit_tail_y(ci, oT, ydst):
                """out-projection + psum copy + ydram store for quarter ci."""
                y_sbq = ysq.tile([P, 4, DIM], BF16, name="y_sbq")
                for qt in range(4):
                    for oh in range(2):
                        y_ps = ps_qy.tile([P, 512], F32, name="y_ps", tag="qy")
                        for kh in range(2):
                            nc.tensor.matmul(
                                y_ps[:],
                                oT[:, kh, qt * P:(qt + 1) * P],
                                wout_sb[:, kh, oh * 512:(oh + 1) * 512],
                                start=(kh == 0), stop=(kh == 1),
                            )
                        if oh == 0:
                            nc.scalar.activation(
                                y_sbq[:, qt, oh * 512:(oh + 1) * 512], y_ps[:],
                                AF.Copy, bias=0.0)
                        else:
                            nc.vector.tensor_copy(
                                y_sbq[:, qt, oh * 512:(oh + 1) * 512], y_ps[:])
                nc.gpsimd.dma_start(ydst, y_sbq[:])

            def ydst_for(ci):
                if ci < 3:
                    return ydram_a[ci * 512:(ci + 1) * 512, :] \
                        .rearrange("(q p) o -> p q o", p=P)
                return ydram_b[:].rearrange("(q p) o -> p q o", p=P)

            pending = None  # (ci, av_pair, sums_t) awaiting tail emission
            oT_prev = None
            for ci in range(NQ):
                if pending is not None:
                    oT_prev = emit_tail_pre(*pending)
                av_pair = [ps_va.tile([P, 2, 256], F32, name=f"av{i}", tag="va")
                           for i in range(2)]
                av_t = [av_pair[qt // 2][:, qt % 2, :] for qt in range(4)]
                sums_t = ps_s.tile([P, 16], F32, name="sums_t", tag="sums", bufs=1)
                # zero accumulator banks via K=1 zero matmuls (see docstring)
                for i in range(2):
                    nc.tensor.matmul(av_pair[i][:], ones1[:], zrow[:],
                                     start=True, stop=True)
                nc.tensor.matmul(sums_t[:], ones1[:], zrow[:, 0:16],
                                 start=True, stop=True)
                for m in range(TT // 2):
                    pts = emit_scores(ci, m)
                    emit_av(m, pts, av_t, sums_t)
                    if m == 1 and pending is not None:
                        emit_tail_y(pending[0], oT_prev, ydst_for(pending[0]))
                        if pending[0] == 2 and not os.environ.get("KNOCOLL"):
                            nc.gpsimd.collective_compute(
                                "ReduceScatter", ALU.add,
                                replica_groups=REPLICA_GROUPS,
                                ins=[ydram_a[:].opt()],
                                outs=[rsout_a[:].opt()],
                            )
                            bna = ysq.tile([P, 3, DIM], BF16, name="bna")
                            OUTCOPY_ENG.dma_start(
                                bna[:], rsout_a[:].rearrange("(c p) o -> p c o", p=P))
                            OUTCOPY_ENG.dma_start(
                                out_ext[0:3 * P, :].rearrange("(c p) o -> p c o", p=P),
                                bna[:])
                        pending = None
                pending = (ci, av_pair, sums_t)

            oT_last = emit_tail_pre(*pending)
            emit_tail_y(pending[0], oT_last, ydst_for(pending[0]))
            if not os.environ.get("KNOCOLL"):
                nc.gpsimd.collective_compute(
                    "ReduceScatter", ALU.add,
                    replica_groups=REPLICA_GROUPS,
                    ins=[ydram_b[:].opt()],
                    outs=[rsout_b[:].opt()],
                )
                bnb = ysq.tile([P, 1, DIM], BF16, name="bnb")
                OUTCOPY_ENG.dma_start(
                    bnb[:], rsout_b[:].rearrange("(c p) o -> p c o", p=P))
                OUTCOPY_ENG.dma_start(
                    out_ext[3 * P:4 * P, :].rearrange("(c p) o -> p c o", p=P),
                    bnb[:])
            else:
                nc.gpsimd.dma_start(out_ext[3 * P:4 * P, :], ydram_b[0:P, :])
                nc.gpsimd.dma_start(out_ext[0:3 * P, :], ydram_a[0:3 * P, :])

    nc.compile()
    return nc


def _get_nc():
    global _nc_cache
    if _nc_cache is None:
        _nc_cache = _build()
    return _nc_cache


_PERM_EO = np.concatenate([np.arange(0, DH, 2), np.arange(1, DH, 2)])


def _shard(core, x, rotary_cos, rotary_sin, gamma, w_qkv, w_gates, b_gates, w_out):
    g, r = core // 4, core % 4
    heads = np.arange(4 * r, 4 * r + 4)
    wq = w_qkv[0 * DIM:1 * DIM] * gamma[None, :]
    wk = w_qkv[1 * DIM:2 * DIM] * gamma[None, :]
    wv = w_qkv[2 * DIM:3 * DIM]

    def qk_rows(w):
        idx = (heads[:, None] * DH + _PERM_EO[None, :]).reshape(-1)
        return w[idx]

    v_rows = wv[(heads[:, None] * DH + np.arange(DH)[None, :]).reshape(-1)]
    wg_rows = w_gates[heads] * gamma[None, :]
    wqkv_t = np.concatenate([qk_rows(wq), qk_rows(wk), v_rows, wg_rows],
                            axis=0).T
    wout_t = w_out[:, heads[0] * DH:heads[0] * DH + HL * DH].T

    cos = rotary_cos[0, 0]  # (N, DH)
    sin = rotary_sin[0, 0]
    cosP = np.tile(np.concatenate([cos[:, 0::2], cos[:, 1::2]], axis=1), (1, 8))
    sinN = np.tile(np.concatenate([-sin[:, 0::2], sin[:, 1::2]], axis=1), (1, 8))

    bf = ml_dtypes.bfloat16
    return {
        "xT": np.ascontiguousarray(x[g].T).astype(bf),
        "xb": np.ascontiguousarray(x[g]).astype(bf),
        "wqkv": np.ascontiguousarray(wqkv_t).astype(bf),
        "wout": np.ascontiguousarray(wout_t).astype(bf),
        "cosP": np.ascontiguousarray(cosP).astype(bf),
        "sinN": np.ascontiguousarray(sinN).astype(bf),
        "bgn": np.tile(b_gates[heads][None, :], (P, 1)).astype(np.float32),
    }


def kernel(x, rotary_cos, rotary_sin, gamma, w_qkv, w_gates, b_gates, w_out):
    global _last_result
    args = [np.asarray(a, np.float32) for a in
            (x, rotary_cos, rotary_sin, gamma, w_qkv, w_gates, b_gates, w_out)]
    nc = _get_nc()
    in_maps = [_shard(c, *args) for c in range(CORES)]
    try:
        res = run_bass_kernel_spmd(
            nc, in_maps, core_ids=list(range(CORES)),
            trace=bool(os.environ.get("KTRACE")),
        )
    except ModuleNotFoundError:
        res = run_bass_kernel_spmd(nc, in_maps, core_ids=list(range(CORES)))
    _last_result = res
    full = np.zeros((B, N, DIM), np.float32)
    for c in range(CORES):
        g, r = c // 4, c % 4
        o = np.asarray(res.results[c]["out"]).astype(np.float32)
        full[g, r * 384:(r + 1) * 384, :] = o[0:384]
        full[g, 1536 + r * P:1536 + (r + 1) * P, :] = o[384:512]
    return full
